# revision 1
# baseline (speedup 1.0000x reference)
"""BiAttentionLayer Trainium2 kernel (Bass/Tile), data-parallel over batch N.

Full inputs:  H [64,1024,200], U [64,64,200], c_mask [64,1024],
              q_mask [64,64], w [600], b []
Full output:  G [64,1024,800] = concat([H, U_, H*U_, H*H_], -1)

Sharding: batch rows 8 per core across 8 NeuronCores; masks/w/b replicated.

Math (matches the reference to fp rounding):
  S = (H@w_h)[:,:,None] + (U@w_u)[:,None,:] + (H*w_hu)@U^T + b
  masked_softmax(v,m) == exp(v*m)*m / sum_j(exp(v*m)*m)   (normalizer of the
  inner softmax cancels on renormalization; the 1e-13 eps is negligible).
  With NEG=100:  e = exp((Sq_cols + (S1+b+NEG))*qm - NEG)  gives the masked
  numerator in one ACT op (masked lanes underflow to ~0), so
    denom_t = sum_j e,   S_t = e/denom,   exp(S_max)*cm = max_j(e)*cm.
  U_ = (e @ U) * (1/denom)  — normalization folded into the PSUM->SBUF copy.
  H_ = (rt @ [H|1]) with rt = max_j(e)*cm, normalized by the ones column.

Schedule: chunk pipeline head(c)=DMA+transpose+S-matmul (PE-dense),
soft(c)=masked softmax (ACT/DVE), tail(c)=e-transpose+U_ matmul; emitted
as head(c+2) / soft(c+1) / tail(c) so ~3 chunks are in flight; the H_
reduction runs as one back-to-back PE burst at row end.
"""

import os
import sys

for _p in ("/opt/trn_rl_repo", "/root/.axon_site/_ro/trn_rl_repo"):
    if os.path.isdir(_p) and _p not in sys.path:
        sys.path.insert(0, _p)

import numpy as np

import concourse.bass as bass
import concourse.tile as tile
from concourse import mybir
from concourse.masks import make_identity

N_CORES = 8
N_FULL = 64
B = N_FULL // N_CORES          # batch rows per core
T = 1024
J = 64
D2 = 200
DG = 4 * D2                    # 800
NCHUNK = T // 128              # 8
K1, K2 = 128, D2 - 128         # contraction split 128 + 72
NEG_SOFT = 100.0               # exp(x - 100): masked lanes underflow to ~0

FP = mybir.dt.float32
FR = mybir.dt.float32r        # single-pass PE matmul; inputs rounded (~tf32)
USE_FP32R = False
MULT = mybir.AluOpType.mult
ADD = mybir.AluOpType.add
AXX = mybir.AxisListType.X
EXP = mybir.ActivationFunctionType.Exp
COPYF = mybir.ActivationFunctionType.Copy


def _split_overwide_waits(nc, max_waits=1):
    """This walrus build only encodes one semaphore wait per instruction;
    hoist extra waits onto no-ops just before the offending instruction."""
    for bb in nc.m.functions[0].blocks:
        i = 0
        while i < len(bb.instructions):
            ins = bb.instructions[i]
            si = getattr(ins, "sync_info", None)
            if si is not None and si.on_wait is not None and len(si.on_wait) > max_waits:
                waits = list(si.on_wait)
                si.on_wait = waits[-max_waits:]
                rest = waits[:-max_waits]
                k = 0
                while rest:
                    chunk, rest = rest[:max_waits], rest[max_waits:]
                    nop = mybir.InstNoOp(
                        name=f"{ins.name}-wsplit{k}",
                        engine=ins.engine,
                        bass_nofuse=True,
                        sync_info=mybir.SyncInfo(on_wait=chunk, on_update=[]),
                    )
                    bb.instructions.insert(i, nop)
                    i += 1
                    k += 1
            i += 1


def build_program(split_waits=True):
    nc = bass.Bass()

    H_d = nc.dram_tensor("H", [B, T, D2], FP, kind="ExternalInput")
    U_d = nc.dram_tensor("U", [B, J, D2], FP, kind="ExternalInput")
    cm_d = nc.dram_tensor("c_mask", [B, T], FP, kind="ExternalInput")
    qm_d = nc.dram_tensor("q_mask", [B, J], FP, kind="ExternalInput")
    w_d = nc.dram_tensor("w", [3 * D2], FP, kind="ExternalInput")
    b_d = nc.dram_tensor("b", [1, 1], FP, kind="ExternalInput")
    G_d = nc.dram_tensor("G", [B, T, DG], FP, kind="ExternalOutput")

    with tile.TileContext(nc) as tc:
        with (
            tc.tile_pool(name="const", bufs=1) as constp,
            tc.tile_pool(name="row", bufs=2) as rowp,
            tc.tile_pool(name="chunk", bufs=6) as chp,
            tc.tile_pool(name="gbuf", bufs=2 * NCHUNK) as gp,
            tc.tile_pool(name="ps_tr", bufs=2, space="PSUM") as ps_trp,
            tc.tile_pool(name="ps_s", bufs=3, space="PSUM") as ps_sp,
            tc.tile_pool(name="ps_u", bufs=3, space="PSUM") as ps_up,
        ):
            # ---- constants ----
            ident = constp.tile([128, 128], FP)
            make_identity(nc, ident)
            ones_row = constp.tile([1, 128], FP)
            nc.vector.memset(ones_row, 1.0)
            negc = constp.tile([128, 1], FP)
            nc.vector.memset(negc, -NEG_SOFT)
            b_sb = constp.tile([1, 1], FP)
            nc.gpsimd.dma_start(out=b_sb, in_=b_d[:, :])
            b100 = constp.tile([1, 1], FP)
            nc.vector.tensor_scalar_add(out=b100, in0=b_sb, scalar1=NEG_SOFT)
            wh1 = constp.tile([K1, 1], FP)
            wh2 = constp.tile([K2, 1], FP)
            wu1 = constp.tile([K1, 1], FP)
            wu2 = constp.tile([K2, 1], FP)
            whu1 = constp.tile([K1, 1], FP)
            whu2 = constp.tile([K2, 1], FP)
            for sb, lo in ((wh1, 0), (wh2, K1), (wu1, D2), (wu2, D2 + K1),
                           (whu1, 2 * D2), (whu2, 2 * D2 + K1)):
                n = sb.shape[0]
                nc.gpsimd.dma_start(out=sb, in_=w_d[lo:lo + n].unsqueeze(1))

            def row_setup(r):
                st = {}
                U_sb = rowp.tile([J, D2], FP, tag="usb")
                nc.sync.dma_start(out=U_sb, in_=U_d[r])
                qm_b = rowp.tile([128, J], FP, tag="qmb")
                nc.gpsimd.dma_start(out=qm_b, in_=qm_d[r].partition_broadcast(128))
                cm_t = rowp.tile([128, NCHUNK], FP, tag="cmt")
                nc.gpsimd.dma_start(
                    out=cm_t, in_=cm_d[r].rearrange("(c p) -> p c", p=128)
                )

                # U^T via PE transpose (two D2 chunks); S2 = U@w_u
                tru = ps_trp.tile([128, 384], FP, tag="tr")
                nc.tensor.transpose(tru[0:K1, 0:J], U_sb[:, 0:K1], ident[0:J, 0:J])
                nc.tensor.transpose(
                    tru[0:K2, J:2 * J], U_sb[:, K1:D2], ident[0:J, 0:J]
                )
                ut1 = rowp.tile([K1, J], FP, tag="ut1")
                ut2 = rowp.tile([K2, J], FP, tag="ut2")
                nc.scalar.copy(out=ut1, in_=tru[0:K1, 0:J])
                nc.scalar.copy(out=ut2, in_=tru[0:K2, J:2 * J])

                # S-matmul rhs: uwq1 [128,65] cols j = U^T*w_hu*qm, col 64 = w_h
                # uwq2 [73,65]: rows 0:72 ditto, row 72 = [S2*qm | b+100],
                # matched by an lhsT ones row produced by transposing the
                # memset ones column g_c[:, 200].
                mmdt = FR if USE_FP32R else FP
                uwq1 = rowp.tile([K1, J + 1], mmdt, tag="uwq1")
                uwq2 = rowp.tile([K2 + 1, J + 1], mmdt, tag="uwq2")
                nc.vector.scalar_tensor_tensor(
                    out=uwq1[:, 0:J], in0=ut1, scalar=whu1[:, 0:1],
                    in1=qm_b[0:K1, :], op0=MULT, op1=MULT,
                )
                nc.vector.scalar_tensor_tensor(
                    out=uwq2[0:K2, 0:J], in0=ut2, scalar=whu2[:, 0:1],
                    in1=qm_b[0:K2, :], op0=MULT, op1=MULT,
                )
                nc.vector.tensor_copy(out=uwq1[:, J:J + 1], in_=wh1)
                nc.vector.tensor_copy(out=uwq2[0:K2, J:J + 1], in_=wh2)

                nc.tensor.matmul(tru[0:J, 128:129], ut1, wu1, start=True, stop=False)
                nc.tensor.matmul(tru[0:J, 128:129], ut2, wu2, start=False, stop=True)
                s2col = rowp.tile([J, 1], FP, tag="s2col")
                nc.vector.tensor_copy(out=s2col, in_=tru[0:J, 128:129])
                nc.tensor.transpose(tru[0:1, 136:200], s2col, ident[0:J, 0:J])
                s2q = rowp.tile([1, J + 1], mmdt, tag="s2q")
                nc.vector.tensor_tensor(
                    out=s2q[:, 0:J], in0=tru[0:1, 136:200],
                    in1=qm_b[0:1, :], op=MULT,
                )
                nc.vector.tensor_copy(out=s2q[:, J:J + 1], in_=b100)
                nc.sync.dma_start(out=uwq2[K2:K2 + 1, :], in_=s2q)

                if USE_FP32R:
                    U_r = rowp.tile([J, D2], FR, tag="ur")
                    nc.vector.tensor_copy(out=U_r, in_=U_sb)
                else:
                    U_r = U_sb
                st["U_r"] = U_r
                st["U_sb"], st["qm_b"], st["cm_t"] = U_sb, qm_b, cm_t
                st["uwq1"], st["uwq2"] = uwq1, uwq2
                denoms = rowp.tile([128, NCHUNK], FP, tag="denoms")
                maxes = rowp.tile([128, NCHUNK], FP, tag="maxes")
                rt = rowp.tile([128, NCHUNK], FP, tag="rt")
                st["denoms"], st["maxes"], st["rt"] = denoms, maxes, rt
                st["g"] = [None] * NCHUNK
                st["ps_s"] = [None] * NCHUNK
                st["e"] = [None] * (NCHUNK // 2)
                st["rden"] = [None] * (NCHUNK // 2)
                return st

            def head(st, r, c):
                t0 = c * 128
                g_c = gp.tile([128, DG], FP, tag="g")
                st["g"][c] = g_c
                nc.sync.dma_start(out=g_c[:, 0:D2], in_=H_d[r, t0:t0 + 128, :])
                # transient ones column: transposed into the lhsT ones row
                # for the S2 rank-1 term; overwritten later by U_
                nc.vector.memset(g_c[:, D2:D2 + 1], 1.0)
                trc = ps_trp.tile([128, 256], FP, tag="tr")
                nc.tensor.transpose(trc[:, 0:128], g_c[:, 0:K1], ident)
                nc.tensor.transpose(
                    trc[0:K2 + 1, 128:256], g_c[:, K1:D2 + 1], ident
                )
                ht = chp.tile([128, 256], FR if USE_FP32R else FP, tag="ht")
                if USE_FP32R:
                    # DVE-produced fp32r operands pass the BIR verifier;
                    # ACT-produced ones were the suspected compile failure
                    nc.vector.tensor_copy(out=ht[:, 0:128], in_=trc[:, 0:128])
                    nc.vector.tensor_copy(
                        out=ht[0:K2 + 1, 128:256], in_=trc[0:K2 + 1, 128:256]
                    )
                else:
                    nc.scalar.copy(out=ht[:, 0:128], in_=trc[:, 0:128])
                    nc.scalar.copy(
                        out=ht[0:K2 + 1, 128:256], in_=trc[0:K2 + 1, 128:256]
                    )
                ps_s = ps_sp.tile([128, J + 1], FP, tag="s")
                st["ps_s"][c] = ps_s
                nc.tensor.matmul(
                    ps_s, ht[:, 0:128], st["uwq1"], start=True, stop=False
                )
                nc.tensor.matmul(
                    ps_s, ht[0:K2 + 1, 128:256], st["uwq2"],
                    start=False, stop=True,
                )

            def soft(st, r, c):
                ps_s = st["ps_s"][c]
                vmq = chp.tile([128, J], FP, tag="vmq")
                nc.vector.scalar_tensor_tensor(
                    out=vmq, in0=ps_s[:, 0:J], scalar=ps_s[:, J:J + 1],
                    in1=st["qm_b"], op0=ADD, op1=MULT,
                )
                if c % 2 == 0:
                    ep = chp.tile([128, 2 * J], FP, tag="e")
                    st["e"][c // 2] = ep
                e_pair = st["e"][c // 2]
                half = (c % 2) * J
                nc.scalar.activation(
                    out=e_pair[:, half:half + J], in_=vmq, func=EXP,
                    bias=negc[:, 0:1], scale=1.0,
                )
                if c % 2 == 1:
                    ep3 = e_pair.rearrange("p (k j) -> p k j", j=J)
                    nc.vector.reduce_sum(
                        st["denoms"][:, c - 1:c + 1], ep3, axis=AXX
                    )
                    nc.vector.reduce_max(
                        st["maxes"][:, c - 1:c + 1], ep3, axis=AXX
                    )
                    rp = chp.tile([128, 2], FP, tag="rden")
                    st["rden"][c // 2] = rp
                    nc.vector.reciprocal(
                        out=rp, in_=st["denoms"][:, c - 1:c + 1]
                    )

            def tail(st, r, c):
                e_pair = st["e"][c // 2]
                half = (c % 2) * J
                g_c = st["g"][c]
                ps_a = ps_up.tile([128, D2], FP, tag="u")
                nc.tensor.transpose(
                    ps_a[0:J, 0:128], e_pair[:, half:half + J], ident
                )
                eT = chp.tile([J, 128], FR if USE_FP32R else FP, tag="eT")
                nc.vector.tensor_copy(out=eT, in_=ps_a[0:J, 0:128])
                ps_b = ps_up.tile([128, D2], FP, tag="u")
                nc.tensor.matmul(ps_b, eT, st["U_r"], start=True, stop=True)
                # U_ = (e@U) * 1/denom, fused into the PSUM->SBUF copy
                rp = st["rden"][c // 2]
                nc.scalar.activation(
                    out=g_c[:, D2:2 * D2], in_=ps_b, func=COPYF,
                    scale=rp[:, c % 2:c % 2 + 1],
                )
                nc.vector.tensor_tensor(
                    out=g_c[:, 2 * D2:3 * D2], in0=g_c[:, 0:D2],
                    in1=g_c[:, D2:2 * D2], op=MULT,
                )

            def rowend(st, r):
                rt = st["rt"]
                nc.vector.tensor_tensor(
                    out=rt, in0=st["maxes"], in1=st["cm_t"], op=MULT
                )
                hbar = ps_up.tile([1, D2], FP, tag="u")
                for c in range(NCHUNK):
                    nc.tensor.matmul(
                        hbar, rt[:, c:c + 1], st["g"][c][:, 0:D2],
                        start=(c == 0), stop=(c == NCHUNK - 1),
                    )
                # rsum = sum(rt): per-partition reduce + transpose + reduce
                rtp = rowp.tile([128, 1], FP, tag="rtp")
                nc.vector.reduce_sum(rtp, rt, axis=AXX)
                trr = ps_trp.tile([1, 128], FP, tag="tr")
                nc.tensor.transpose(trr, rtp, ident)
                rtr = rowp.tile([1, 128], FP, tag="rtr")
                nc.vector.tensor_copy(out=rtr, in_=trr)
                rs = rowp.tile([1, 1], FP, tag="rs")
                nc.vector.reduce_sum(rs, rtr, axis=AXX)
                nc.vector.tensor_scalar_add(out=rs, in0=rs, scalar1=1e-13)
                nc.vector.reciprocal(out=rs, in_=rs)
                hbar_sb = rowp.tile([1, D2], FP, tag="hbar_sb")
                nc.vector.tensor_scalar_mul(
                    out=hbar_sb, in0=hbar[:, 0:D2], scalar1=rs[:, 0:1]
                )
                ps_hb = ps_up.tile([128, D2], FP, tag="u")
                nc.tensor.matmul(ps_hb, ones_row, hbar_sb, start=True, stop=True)
                hb_sb = rowp.tile([128, D2], FP, tag="hb_sb")
                nc.vector.tensor_copy(out=hb_sb, in_=ps_hb)
                st["hb_sb"] = hb_sb

            def rowfin(st, r, c):
                g_c = st["g"][c]
                nc.gpsimd.tensor_mul(
                    g_c[:, 3 * D2:4 * D2], g_c[:, 0:D2], st["hb_sb"]
                )
                t0 = c * 128
                nc.sync.dma_start(out=G_d[r, t0:t0 + 128, :], in_=g_c)

            # ---- cross-row pipelined schedule ----
            states = [None] * B
            states[0] = row_setup(0)
            head(states[0], 0, 0)
            head(states[0], 0, 1)
            head(states[0], 0, 2)
            soft(states[0], 0, 0)
            for r in range(B):
                st = states[r]
                prev = states[r - 1] if r > 0 else None
                for c in range(NCHUNK):
                    if c + 1 < NCHUNK:
                        soft(st, r, c + 1)
                    tail(st, r, c)
                    if c + 3 < NCHUNK:
                        head(st, r, c + 3)
                    # previous row's H*H_ and store, spread across this row's
                    # iterations so loads/stores interleave on the SP ring
                    if prev is not None:
                        rowfin(prev, r - 1, c)
                    if c == 3 and r + 1 < B:
                        states[r + 1] = row_setup(r + 1)
                if r + 1 < B:
                    nxt = states[r + 1]
                    head(nxt, r + 1, 0)
                    head(nxt, r + 1, 1)
                rowend(st, r)
                if r + 1 < B:
                    nxt = states[r + 1]
                    head(nxt, r + 1, 2)
                    soft(nxt, r + 1, 0)
            for c in range(NCHUNK):
                rowfin(states[B - 1], B - 1, c)

    if split_waits:
        _split_overwide_waits(nc)
    return nc


_NC_CACHE = None


def _get_nc():
    global _NC_CACHE
    if _NC_CACHE is None:
        _NC_CACHE = build_program()
    return _NC_CACHE


def run_sharded(inputs, trace=False):
    from concourse.bass_utils import run_bass_kernel_spmd

    H = np.ascontiguousarray(np.asarray(inputs["H"], dtype=np.float32))
    U = np.ascontiguousarray(np.asarray(inputs["U"], dtype=np.float32))
    cm = np.ascontiguousarray(np.asarray(inputs["c_mask"], dtype=np.float32))
    qm = np.ascontiguousarray(np.asarray(inputs["q_mask"], dtype=np.float32))
    w = np.ascontiguousarray(np.asarray(inputs["w"], dtype=np.float32))
    b = np.asarray(inputs["b"], dtype=np.float32).reshape(1, 1)

    nc = _get_nc()
    in_maps = []
    for c in range(N_CORES):
        s = slice(c * B, (c + 1) * B)
        in_maps.append(
            {"H": H[s], "U": U[s], "c_mask": cm[s], "q_mask": qm[s], "w": w, "b": b}
        )
    res = run_bass_kernel_spmd(
        nc, in_maps, core_ids=list(range(N_CORES)), trace=trace
    )
    G = np.concatenate([res.results[c]["G"] for c in range(N_CORES)], axis=0)
    return G, res


def kernel(H, U, c_mask, q_mask, w, b):
    G, _ = run_sharded(
        {"H": H, "U": U, "c_mask": c_mask, "q_mask": q_mask, "w": w, "b": b}
    )
    return G



# revision 6
# speedup vs baseline: 1.1788x; 1.1788x over previous
"""BiAttentionLayer Trainium2 kernel (Bass/Tile), data-parallel over batch N.

Full inputs:  H [64,1024,200], U [64,64,200], c_mask [64,1024],
              q_mask [64,64], w [600], b []
Full output:  G [64,1024,800] = concat([H, U_, H*U_, H*H_], -1)

Sharding: batch rows 8 per core across 8 NeuronCores; masks/w/b replicated.

Math (matches the reference to bf16 rounding; gate is rel_err < 2e-2):
  S = (H@w_h)[:,:,None] + (U@w_u)[:,None,:] + (H*w_hu)@U^T + b
  masked_softmax(v,m) == exp(v*m)*m / sum_j(...) and the C2Q normalization is
  invariant to any per-t factor, so with NEG=100:
    e[t,j] = exp((S[t,j]+100)*qm[j] - 100)  (masked lanes underflow to ~0)
    U_ = (e @ U) / sum_j e,   rt = max_j(e)*cm,  a = rt/sum_t rt, H_ = a@H.

This version computes S TRANSPOSED: S'[j,t] = sum_d uwq[d,j] * H^T[d,t] with
  uwq[d,j] = (w_hu[d]*U[j,d] + w_h[d]) * qm[j]          (folds S1, S3, mask)
  bias[j]  = (S2[j] + b + 100) * qm[j] - 100            (ACT exp bias column)
so e'[j,t] = exp(S' + bias) comes out of ONE activation op with no separate
mask/add pass, and e' is directly the lhsT of the U_ matmul (no e-transpose
before the matmul; a cheap PE transpose recovers [t,j] just for the row max).
All PE operands are bf16 (4x matmul-cycle + 2x weight-load win vs fp32); PSUM
accumulation stays fp32. H/G/U_ stay fp32 end to end in SBUF/HBM.

DMA is batched per batch-row: ONE 819KB load of H[r] into the row tile's
H columns and ONE 3.27MB store of the full G row, so HWDGE issue overhead
(~0.6us per dma_start) stops mattering; engines see ~93us of HBM traffic
per core which is the roofline for this memory-bound problem.
"""

import os
import sys

for _p in ("/opt/trn_rl_repo", "/root/.axon_site/_ro/trn_rl_repo"):
    if os.path.isdir(_p) and _p not in sys.path:
        sys.path.insert(0, _p)

import numpy as np

import concourse.bass as bass
import concourse.tile as tile
from concourse import mybir
from concourse.masks import make_identity

N_CORES = 8
N_FULL = 64
B = N_FULL // N_CORES          # batch rows per core
T = 1024
J = 64
D2 = 200
DG = 4 * D2                    # 800
NCHUNK = T // 128              # 8
K1, K2 = 128, D2 - 128         # contraction split 128 + 72
NEG_SOFT = 100.0               # exp(x - 100): masked lanes underflow to ~0

FP = mybir.dt.float32
BF = mybir.dt.bfloat16
MULT = mybir.AluOpType.mult
ADD = mybir.AluOpType.add
AXX = mybir.AxisListType.X
EXP = mybir.ActivationFunctionType.Exp
COPYF = mybir.ActivationFunctionType.Copy


def _split_overwide_waits(nc, max_waits=1):
    """This walrus build only encodes one semaphore wait per instruction;
    hoist extra waits onto no-ops just before the offending instruction."""
    for bb in nc.m.functions[0].blocks:
        i = 0
        while i < len(bb.instructions):
            ins = bb.instructions[i]
            si = getattr(ins, "sync_info", None)
            if si is not None and si.on_wait is not None and len(si.on_wait) > max_waits:
                waits = list(si.on_wait)
                si.on_wait = waits[-max_waits:]
                rest = waits[:-max_waits]
                k = 0
                while rest:
                    chunk, rest = rest[:max_waits], rest[max_waits:]
                    nop = mybir.InstNoOp(
                        name=f"{ins.name}-wsplit{k}",
                        engine=ins.engine,
                        bass_nofuse=True,
                        sync_info=mybir.SyncInfo(on_wait=chunk, on_update=[]),
                    )
                    bb.instructions.insert(i, nop)
                    i += 1
                    k += 1
            i += 1


def build_program(split_waits=True):
    nc = bass.Bass()

    H_d = nc.dram_tensor("H", [B, T, D2], FP, kind="ExternalInput")
    U_d = nc.dram_tensor("U", [B, J, D2], FP, kind="ExternalInput")
    cm_d = nc.dram_tensor("c_mask", [B, T], FP, kind="ExternalInput")
    qm_d = nc.dram_tensor("q_mask", [B, J], FP, kind="ExternalInput")
    w_d = nc.dram_tensor("w", [3 * D2], FP, kind="ExternalInput")
    b_d = nc.dram_tensor("b", [1, 1], FP, kind="ExternalInput")
    G_d = nc.dram_tensor("G", [B, T, DG], FP, kind="ExternalOutput")

    with tile.TileContext(nc) as tc:
        with (
            tc.tile_pool(name="const", bufs=1) as constp,
            tc.tile_pool(name="row", bufs=2) as rowp,
            tc.tile_pool(name="grow", bufs=3) as growp,
            tc.tile_pool(name="hb", bufs=14) as hbp,
            tc.tile_pool(name="chunk", bufs=3) as chp,
            tc.tile_pool(name="ps_tr", bufs=2, space="PSUM") as ptrp,
            tc.tile_pool(name="ps_s", bufs=2, space="PSUM") as ps_sp,
            tc.tile_pool(name="ps_u", bufs=2, space="PSUM") as ps_up,
            tc.tile_pool(name="ps_e", bufs=1, space="PSUM") as ps_ep,
            tc.tile_pool(name="ps_row", bufs=1, space="PSUM") as rowps,
        ):
            # ---- constants ----
            identf = constp.tile([128, 128], FP)
            make_identity(nc, identf)
            identb = constp.tile([128, 128], BF)
            nc.vector.tensor_copy(out=identb, in_=identf)
            ones_row = constp.tile([1, 128], BF)
            nc.vector.memset(ones_row, 1.0)

            b64 = constp.tile([J, 1], FP)
            nc.gpsimd.dma_start(out=b64, in_=b_d[:, :].partition_broadcast(J))
            b100 = constp.tile([J, 1], FP)
            nc.vector.tensor_scalar_add(out=b100, in0=b64, scalar1=NEG_SOFT)

            # w pieces: f32 columns for the uwq build, bf16 for the S2 matmul
            wh1 = constp.tile([K1, 1], FP)
            wh2 = constp.tile([K2, 1], FP)
            whu1 = constp.tile([K1, 1], FP)
            whu2 = constp.tile([K2, 1], FP)
            wu1b = constp.tile([K1, 1], BF)
            wu2b = constp.tile([K2, 1], BF)
            for sb, lo in ((wh1, 0), (wh2, K1), (whu1, 2 * D2), (whu2, 2 * D2 + K1),
                           (wu1b, D2), (wu2b, D2 + K1)):
                n = sb.shape[0]
                nc.gpsimd.dma_start(out=sb, in_=w_d[lo:lo + n].unsqueeze(1))

            # batched per-problem loads (all rows at once)
            qm_b = constp.tile([128, B * J], BF)     # [p, r*64+j] = qm[r, j]
            nc.gpsimd.dma_start(
                out=qm_b, in_=qm_d.rearrange("r j -> (r j)").partition_broadcast(128)
            )
            qm_col = constp.tile([J, B], FP)         # [j, r]
            nc.gpsimd.dma_start(out=qm_col, in_=qm_d.rearrange("r j -> j r"))
            cm8 = constp.tile([B, T], FP)            # raw [r, t]
            nc.sync.dma_start(out=cm8, in_=cm_d[:, :])
            U_all = constp.tile([J, B * D2], FP)     # [j, r*200+d]
            nc.sync.dma_start(
                out=U_all.rearrange("j (r d) -> j r d", d=D2),
                in_=U_d.rearrange("r j d -> j r d"),
            )

            # cmT[p, c*8+r] = c_mask[r, c*128+p]  via 8 small PE transposes
            cmT = constp.tile([128, NCHUNK * B], BF)
            for c in range(NCHUNK):
                cmps = rowps.tile([128, B], FP, tag="row")
                nc.tensor.transpose(
                    cmps, cm8[:, c * 128:(c + 1) * 128], identf[0:B, 0:B]
                )
                nc.vector.tensor_copy(out=cmT[:, c * B:(c + 1) * B], in_=cmps)

            def load_grow(r):
                g = growp.tile([128, NCHUNK * DG], FP, tag="g")
                gv = g.rearrange("p (c gg) -> p c gg", gg=DG)
                nc.sync.dma_start(
                    out=gv[:, :, 0:D2],
                    in_=H_d[r].rearrange("(c p) d -> p c d", p=128),
                )
                return g

            def store_grow(r, g):
                nc.sync.dma_start(
                    out=G_d[r].rearrange("(c p) gg -> p c gg", p=128),
                    in_=g.rearrange("p (c gg) -> p c gg", gg=DG),
                )

            def row_setup(r):
                st = {"r": r}
                Ub = rowp.tile([J, D2 + 1], BF, tag="ub")
                nc.scalar.copy(out=Ub[:, 0:D2], in_=U_all[:, r * D2:(r + 1) * D2])
                nc.vector.memset(Ub[:, D2:D2 + 1], 1.0)
                UTps = rowps.tile([128, 128], BF, tag="row")
                nc.tensor.transpose(UTps[:, 0:J], Ub[:, 0:K1], identb[0:J, 0:J])
                nc.tensor.transpose(
                    UTps[0:K2, J:2 * J], Ub[:, K1:D2], identb[0:J, 0:J]
                )
                UT = rowp.tile([128, 128], BF, tag="ut")
                nc.vector.tensor_copy(out=UT[:, 0:J], in_=UTps[:, 0:J])
                nc.vector.tensor_copy(
                    out=UT[0:K2, J:2 * J], in_=UTps[0:K2, J:2 * J]
                )

                uwq1 = rowp.tile([K1, J], BF, tag="uwq1")
                uwq2 = rowp.tile([K2, J], BF, tag="uwq2")
                nc.vector.tensor_scalar(
                    out=uwq1, in0=UT[:, 0:J], scalar1=whu1[:, 0:1],
                    scalar2=wh1[:, 0:1], op0=MULT, op1=ADD,
                )
                nc.vector.tensor_tensor(
                    out=uwq1, in0=uwq1, in1=qm_b[:, r * J:(r + 1) * J], op=MULT
                )
                nc.vector.tensor_scalar(
                    out=uwq2, in0=UT[0:K2, J:2 * J], scalar1=whu2[:, 0:1],
                    scalar2=wh2[:, 0:1], op0=MULT, op1=ADD,
                )
                nc.vector.tensor_tensor(
                    out=uwq2, in0=uwq2, in1=qm_b[0:K2, r * J:(r + 1) * J], op=MULT
                )

                S2ps = rowps.tile([J, 1], FP, tag="row")
                nc.tensor.matmul(S2ps, UT[:, 0:J], wu1b, start=True, stop=False)
                nc.tensor.matmul(
                    S2ps, UT[0:K2, J:2 * J], wu2b, start=False, stop=True
                )
                bias = rowp.tile([J, 1], FP, tag="bias")
                nc.vector.scalar_tensor_tensor(
                    out=bias, in0=S2ps, scalar=b100[:, 0:1],
                    in1=qm_col[:, r:r + 1], op0=ADD, op1=MULT,
                )
                nc.vector.tensor_scalar_add(out=bias, in0=bias, scalar1=-NEG_SOFT)

                st["Ub"], st["uwq1"], st["uwq2"], st["bias"] = Ub, uwq1, uwq2, bias
                rt = rowp.tile([128, NCHUNK], BF, tag="rt")
                st["rt"] = rt
                st["Hb"] = [None] * NCHUNK
                st["ps_s"] = [None] * NCHUNK
                st["eT"] = [None] * NCHUNK
                return st

            def head(st, c):
                g = st["g"]
                Hb = hbp.tile([128, D2], BF, tag="hb")
                st["Hb"][c] = Hb
                nc.scalar.copy(out=Hb, in_=g[:, c * DG:c * DG + D2])
                trc = ptrp.tile([128, 256], BF, tag="tr")
                nc.tensor.transpose(trc[:, 0:128], Hb[:, 0:K1], identb)
                nc.tensor.transpose(trc[0:K2, 128:256], Hb[:, K1:D2], identb)
                ht = chp.tile([128, 256], BF, tag="ht")
                nc.vector.tensor_copy(out=ht[:, 0:128], in_=trc[:, 0:128])
                nc.vector.tensor_copy(
                    out=ht[0:K2, 128:256], in_=trc[0:K2, 128:256]
                )
                ps_s = ps_sp.tile([J, 128], FP, tag="s")
                st["ps_s"][c] = ps_s
                nc.tensor.matmul(
                    ps_s, st["uwq1"], ht[:, 0:128], start=True, stop=False
                )
                nc.tensor.matmul(
                    ps_s, st["uwq2"], ht[0:K2, 128:256], start=False, stop=True
                )

            def soft(st, c):
                eT = chp.tile([J, 128], BF, tag="eT")
                st["eT"][c] = eT
                nc.scalar.activation(
                    out=eT, in_=st["ps_s"][c], func=EXP,
                    bias=st["bias"][:, 0:1], scale=1.0,
                )

            def tail(st, c):
                r = st["r"]
                g = st["g"]
                eT = st["eT"][c]
                psU = ps_up.tile([128, D2 + 1], FP, tag="u")
                nc.tensor.matmul(psU, eT, st["Ub"], start=True, stop=True)
                eP = ps_ep.tile([128, J], BF, tag="e")
                nc.tensor.transpose(eP, eT, identb[0:J, 0:J])
                rp = chp.tile([128, 1], FP, tag="rp")
                nc.vector.reciprocal(out=rp, in_=psU[:, D2:D2 + 1])
                nc.scalar.activation(
                    out=g[:, c * DG + D2:c * DG + 2 * D2], in_=psU[:, 0:D2],
                    func=COPYF, scale=rp[:, 0:1],
                )
                rt = st["rt"]
                nc.vector.reduce_max(rt[:, c:c + 1], eP, axis=AXX)
                nc.vector.tensor_tensor(
                    out=rt[:, c:c + 1], in0=rt[:, c:c + 1],
                    in1=cmT[:, c * B + r:c * B + r + 1], op=MULT,
                )
                nc.vector.tensor_tensor(
                    out=g[:, c * DG + 2 * D2:c * DG + 3 * D2],
                    in0=g[:, c * DG:c * DG + D2],
                    in1=g[:, c * DG + D2:c * DG + 2 * D2], op=MULT,
                )

            def rowend(st):
                rt = st["rt"]
                ps_h = rowps.tile([128, 2], FP, tag="row")
                for c in range(NCHUNK):
                    nc.tensor.matmul(
                        ps_h[:, 0:1], st["Hb"][c][:, 0:K1], rt[:, c:c + 1],
                        start=(c == 0), stop=(c == NCHUNK - 1),
                    )
                for c in range(NCHUNK):
                    nc.tensor.matmul(
                        ps_h[0:K2, 1:2], st["Hb"][c][:, K1:D2], rt[:, c:c + 1],
                        start=(c == 0), stop=(c == NCHUNK - 1),
                    )
                rtp = rowp.tile([128, 1], FP, tag="rtp")
                nc.vector.reduce_sum(rtp, rt, axis=AXX)
                hbc = rowp.tile([128, 2], BF, tag="hbc")
                nc.vector.tensor_copy(out=hbc[:, 0:1], in_=ps_h[:, 0:1])
                nc.vector.tensor_copy(out=hbc[0:K2, 1:2], in_=ps_h[0:K2, 1:2])
                rtpT = rowps.tile([1, 128], FP, tag="row")
                nc.tensor.transpose(rtpT, rtp, identf)
                rtr = rowp.tile([1, 128], FP, tag="rtr")
                nc.vector.tensor_copy(out=rtr, in_=rtpT)
                rs = rowp.tile([1, 1], FP, tag="rs")
                nc.vector.reduce_sum(rs, rtr, axis=AXX)
                nc.vector.tensor_scalar_add(out=rs, in0=rs, scalar1=1e-13)
                nc.vector.reciprocal(out=rs, in_=rs)
                hbrow = rowps.tile([1, D2], BF, tag="row")
                nc.tensor.transpose(hbrow[0:1, 0:K1], hbc[:, 0:1], identb)
                nc.tensor.transpose(
                    hbrow[0:1, K1:D2], hbc[0:K2, 1:2], identb[0:K2, 0:K2]
                )
                hbar_sb = rowp.tile([1, D2], BF, tag="hbar_sb")
                nc.vector.tensor_scalar_mul(
                    out=hbar_sb, in0=hbrow, scalar1=rs[:, 0:1]
                )
                psb = rowps.tile([128, D2], FP, tag="row")
                nc.tensor.matmul(psb, ones_row, hbar_sb, start=True, stop=True)
                hb_sb = rowp.tile([128, D2], FP, tag="hb_sb")
                nc.vector.tensor_copy(out=hb_sb, in_=psb)
                st["hb_sb"] = hb_sb

            def rowfin(st, c):
                g = st["g"]
                nc.gpsimd.tensor_mul(
                    g[:, c * DG + 3 * D2:c * DG + 4 * D2],
                    g[:, c * DG:c * DG + D2], st["hb_sb"]
                )

            # ---- cross-row pipelined schedule ----
            grows = [None] * B
            for r in range(min(3, B)):
                grows[r] = load_grow(r)
            states = [None] * B
            states[0] = row_setup(0)
            states[0]["g"] = grows[0]
            head(states[0], 0)
            head(states[0], 1)
            soft(states[0], 0)
            for r in range(B):
                st = states[r]
                prev = states[r - 1] if r > 0 else None
                for c in range(NCHUNK):
                    if c + 1 < NCHUNK:
                        soft(st, c + 1)
                    tail(st, c)
                    if c + 2 < NCHUNK:
                        head(st, c + 2)
                    # previous row's H*H_, spread across this row's iterations
                    if prev is not None:
                        rowfin(prev, c)
                    if c == 3 and r + 1 < B:
                        states[r + 1] = row_setup(r + 1)
                        states[r + 1]["g"] = grows[r + 1]
                    if c == NCHUNK - 1:
                        if prev is not None:
                            store_grow(r - 1, prev["g"])
                        if r + 2 < B and grows[r + 2] is None:
                            grows[r + 2] = load_grow(r + 2)
                if r + 1 < B:
                    nxt = states[r + 1]
                    head(nxt, 0)
                    head(nxt, 1)
                rowend(st)
                if r + 1 < B:
                    soft(states[r + 1], 0)
            for c in range(NCHUNK):
                rowfin(states[B - 1], c)
            store_grow(B - 1, states[B - 1]["g"])

    if split_waits:
        _split_overwide_waits(nc)
    return nc


_NC_CACHE = None


def _get_nc():
    global _NC_CACHE
    if _NC_CACHE is None:
        _NC_CACHE = build_program()
    return _NC_CACHE


def run_sharded(inputs, trace=False):
    from concourse.bass_utils import run_bass_kernel_spmd

    H = np.ascontiguousarray(np.asarray(inputs["H"], dtype=np.float32))
    U = np.ascontiguousarray(np.asarray(inputs["U"], dtype=np.float32))
    cm = np.ascontiguousarray(np.asarray(inputs["c_mask"], dtype=np.float32))
    qm = np.ascontiguousarray(np.asarray(inputs["q_mask"], dtype=np.float32))
    w = np.ascontiguousarray(np.asarray(inputs["w"], dtype=np.float32))
    b = np.asarray(inputs["b"], dtype=np.float32).reshape(1, 1)

    nc = _get_nc()
    in_maps = []
    for c in range(N_CORES):
        s = slice(c * B, (c + 1) * B)
        in_maps.append(
            {"H": H[s], "U": U[s], "c_mask": cm[s], "q_mask": qm[s], "w": w, "b": b}
        )
    res = run_bass_kernel_spmd(
        nc, in_maps, core_ids=list(range(N_CORES)), trace=trace
    )
    G = np.concatenate([res.results[c]["G"] for c in range(N_CORES)], axis=0)
    return G, res


def kernel(H, U, c_mask, q_mask, w, b):
    G, _ = run_sharded(
        {"H": H, "U": U, "c_mask": c_mask, "q_mask": q_mask, "w": w, "b": b}
    )
    return G


# revision 9
# speedup vs baseline: 1.1859x; 1.0060x over previous
"""BiAttentionLayer Trainium2 kernel (Bass/Tile), data-parallel over batch N.

Full inputs:  H [64,1024,200], U [64,64,200], c_mask [64,1024],
              q_mask [64,64], w [600], b []
Full output:  G [64,1024,800] = concat([H, U_, H*U_, H*H_], -1)

Sharding: batch rows 8 per core across 8 NeuronCores; masks/w/b replicated.

Math (matches the reference to bf16 rounding; gate is rel_err < 2e-2):
  S = (H@w_h)[:,:,None] + (U@w_u)[:,None,:] + (H*w_hu)@U^T + b
  masked_softmax(v,m) == exp(v*m)*m / sum_j(...); the C2Q normalization is
  invariant to any per-t factor, so with NEG=100:
    e[t,j] = exp((S[t,j]+100)*qm[j] - 100)  (masked lanes underflow to ~0)
    U_ = (e @ U) / sum_j e,   rt = max_j(e)*cm,  a = rt/sum_t rt, H_ = a@H.

S is computed TRANSPOSED: S'[j,t] = sum_d uwq[d,j] * H^T[d,t] with
  uwq[d,j] = (w_hu[d]*U[j,d] + w_h[d]) * qm[j]          (folds S1, S3, mask)
  bias[j]  = (S2[j] + b + 100) * qm[j] - 100            (ACT exp bias column)
so e'[j,t] = exp(S' + bias) is ONE activation op and e' is directly the lhsT
of the U_ matmul; a small PE transpose recovers [t,j] just for the row max.
All PE operands are bf16; PSUM accumulation and H/U_/G stay fp32.

This revision is tuned for per-instruction overhead (the engines were all
~55% busy but latency-bound): chunks are processed in PAIRS so each DVE/ACT
instruction covers 2 chunks (Hb cast, ht copy, exp, reciprocal, U_ scale,
reduce_max, H*U_; H*H_ covers 4), the contraction is split 100+100 so one
copy drains both transposes, the U_ normalization runs on DVE as a
tensor_tensor against a stride-0 broadcast of the reciprocal pair, and DMA
is one 819KB load + one 3.27MB store per batch row (~93us/core of HBM
traffic = the roofline for this memory-bound problem).
"""

import os
import sys

for _p in ("/opt/trn_rl_repo", "/root/.axon_site/_ro/trn_rl_repo"):
    if os.path.isdir(_p) and _p not in sys.path:
        sys.path.insert(0, _p)

import numpy as np

import concourse.bass as bass
import concourse.tile as tile
from concourse import mybir
from concourse.masks import make_identity

N_CORES = 8
N_FULL = 64
B = N_FULL // N_CORES          # batch rows per core
T = 1024
J = 64
D2 = 200
DG = 4 * D2                    # 800
NCHUNK = T // 128              # 8
NPAIR = NCHUNK // 2            # 4
KH = 100                       # contraction split 100 + 100
NEG_SOFT = 100.0               # exp(x - 100): masked lanes underflow to ~0

FP = mybir.dt.float32
BF = mybir.dt.bfloat16
MULT = mybir.AluOpType.mult
ADD = mybir.AluOpType.add
AXX = mybir.AxisListType.X
EXP = mybir.ActivationFunctionType.Exp
COPYF = mybir.ActivationFunctionType.Copy


def _split_overwide_waits(nc, max_waits=1):
    """This walrus build only encodes one semaphore wait per instruction;
    hoist extra waits onto no-ops just before the offending instruction."""
    for bb in nc.m.functions[0].blocks:
        i = 0
        while i < len(bb.instructions):
            ins = bb.instructions[i]
            si = getattr(ins, "sync_info", None)
            if si is not None and si.on_wait is not None and len(si.on_wait) > max_waits:
                waits = list(si.on_wait)
                si.on_wait = waits[-max_waits:]
                rest = waits[:-max_waits]
                k = 0
                while rest:
                    chunk, rest = rest[:max_waits], rest[max_waits:]
                    nop = mybir.InstNoOp(
                        name=f"{ins.name}-wsplit{k}",
                        engine=ins.engine,
                        bass_nofuse=True,
                        sync_info=mybir.SyncInfo(on_wait=chunk, on_update=[]),
                    )
                    bb.instructions.insert(i, nop)
                    i += 1
                    k += 1
            i += 1


def build_program(split_waits=True):
    nc = bass.Bass()

    H_d = nc.dram_tensor("H", [B, T, D2], FP, kind="ExternalInput")
    U_d = nc.dram_tensor("U", [B, J, D2], FP, kind="ExternalInput")
    cm_d = nc.dram_tensor("c_mask", [B, T], FP, kind="ExternalInput")
    qm_d = nc.dram_tensor("q_mask", [B, J], FP, kind="ExternalInput")
    w_d = nc.dram_tensor("w", [3 * D2], FP, kind="ExternalInput")
    b_d = nc.dram_tensor("b", [1, 1], FP, kind="ExternalInput")
    G_d = nc.dram_tensor("G", [B, T, DG], FP, kind="ExternalOutput")

    with tile.TileContext(nc) as tc:
        with (
            tc.tile_pool(name="const", bufs=1) as constp,
            tc.tile_pool(name="row", bufs=2) as rowp,
            tc.tile_pool(name="grow", bufs=3) as growp,
            tc.tile_pool(name="hb", bufs=7) as hbp,
            tc.tile_pool(name="chunk", bufs=3) as chp,
            tc.tile_pool(name="ps_tr", bufs=2, space="PSUM") as ptrp,
            tc.tile_pool(name="ps_s", bufs=2, space="PSUM") as ps_sp,
            tc.tile_pool(name="ps_u", bufs=2, space="PSUM") as ps_up,
            tc.tile_pool(name="ps_e", bufs=1, space="PSUM") as ps_ep,
            tc.tile_pool(name="ps_row", bufs=1, space="PSUM") as rowps,
        ):
            # ---- constants ----
            identf = constp.tile([128, 128], FP)
            make_identity(nc, identf)
            identb = constp.tile([128, 128], BF)
            nc.vector.tensor_copy(out=identb, in_=identf)
            ones_row = constp.tile([1, 128], BF)
            nc.vector.memset(ones_row, 1.0)

            b64 = constp.tile([J, 1], FP)
            nc.gpsimd.dma_start(out=b64, in_=b_d[:, :].partition_broadcast(J))
            b100 = constp.tile([J, 1], FP)
            nc.vector.tensor_scalar_add(out=b100, in0=b64, scalar1=NEG_SOFT)

            # w pieces on the d-split: wh/whu f32 for the uwq build,
            # wu bf16 for the S2 matmul
            wh1 = constp.tile([KH, 1], FP)
            wh2 = constp.tile([KH, 1], FP)
            whu1 = constp.tile([KH, 1], FP)
            whu2 = constp.tile([KH, 1], FP)
            wu1b = constp.tile([KH, 1], BF)
            wu2b = constp.tile([KH, 1], BF)
            for sb, lo in ((wh1, 0), (wh2, KH), (whu1, 2 * D2), (whu2, 2 * D2 + KH),
                           (wu1b, D2), (wu2b, D2 + KH)):
                nc.gpsimd.dma_start(out=sb, in_=w_d[lo:lo + KH].unsqueeze(1))

            # batched per-problem loads (all rows at once)
            qm_b = constp.tile([128, B * J], BF)     # [p, r*64+j] = qm[r, j]
            nc.gpsimd.dma_start(
                out=qm_b, in_=qm_d.rearrange("r j -> (r j)").partition_broadcast(128)
            )
            qm_col = constp.tile([J, B], FP)         # [j, r]
            nc.gpsimd.dma_start(out=qm_col, in_=qm_d.rearrange("r j -> j r"))
            cm8 = constp.tile([B, T], FP)            # raw [r, t]
            nc.sync.dma_start(out=cm8, in_=cm_d[:, :])
            U_all = constp.tile([J, B * D2], FP)     # [j, r*200+d]
            nc.sync.dma_start(
                out=U_all.rearrange("j (r d) -> j r d", d=D2),
                in_=U_d.rearrange("r j d -> j r d"),
            )

            # cmT[p, c*8+r] = c_mask[r, c*128+p]  via 8 small PE transposes
            cmT = constp.tile([128, NCHUNK * B], BF)
            for c in range(NCHUNK):
                cmps = rowps.tile([128, B], FP, tag="row")
                nc.tensor.transpose(
                    cmps, cm8[:, c * 128:(c + 1) * 128], identf[0:B, 0:B]
                )
                nc.vector.tensor_copy(out=cmT[:, c * B:(c + 1) * B], in_=cmps)

            def load_grow(r):
                g = growp.tile([128, NCHUNK * DG], FP, tag="g")
                gv = g.rearrange("p (c gg) -> p c gg", gg=DG)
                nc.sync.dma_start(
                    out=gv[:, :, 0:D2],
                    in_=H_d[r].rearrange("(c p) d -> p c d", p=128),
                )
                return g

            def store_grow(r, g):
                nc.sync.dma_start(
                    out=G_d[r].rearrange("(c p) gg -> p c gg", p=128),
                    in_=g.rearrange("p (c gg) -> p c gg", gg=DG),
                )

            def row_setup(r):
                st = {"r": r}
                Ub = rowp.tile([J, D2 + 1], BF, tag="ub")
                nc.scalar.copy(out=Ub[:, 0:D2], in_=U_all[:, r * D2:(r + 1) * D2])
                nc.vector.memset(Ub[:, D2:D2 + 1], 1.0)
                UTps = rowps.tile([KH, 128], BF, tag="row")
                nc.tensor.transpose(UTps[:, 0:J], Ub[:, 0:KH], identb[0:J, 0:J])
                nc.tensor.transpose(
                    UTps[:, J:2 * J], Ub[:, KH:D2], identb[0:J, 0:J]
                )
                UT = rowp.tile([KH, 128], BF, tag="ut")
                nc.vector.tensor_copy(out=UT, in_=UTps)

                # uwq[:, 0:64] = block d=0:100, uwq[:, 64:128] = block d=100:200
                uwq = rowp.tile([KH, 128], BF, tag="uwq")
                nc.vector.tensor_scalar(
                    out=uwq[:, 0:J], in0=UT[:, 0:J], scalar1=whu1[:, 0:1],
                    scalar2=wh1[:, 0:1], op0=MULT, op1=ADD,
                )
                nc.vector.tensor_scalar(
                    out=uwq[:, J:2 * J], in0=UT[:, J:2 * J], scalar1=whu2[:, 0:1],
                    scalar2=wh2[:, 0:1], op0=MULT, op1=ADD,
                )
                qslice = qm_b[0:KH, r * J:(r + 1) * J]
                nc.vector.tensor_tensor(
                    out=uwq[:, 0:J], in0=uwq[:, 0:J], in1=qslice, op=MULT
                )
                nc.vector.tensor_tensor(
                    out=uwq[:, J:2 * J], in0=uwq[:, J:2 * J], in1=qslice, op=MULT
                )

                S2ps = rowps.tile([J, 1], FP, tag="row")
                nc.tensor.matmul(S2ps, UT[:, 0:J], wu1b, start=True, stop=False)
                nc.tensor.matmul(S2ps, UT[:, J:2 * J], wu2b, start=False, stop=True)
                bias = rowp.tile([J, 1], FP, tag="bias")
                nc.vector.scalar_tensor_tensor(
                    out=bias, in0=S2ps, scalar=b100[:, 0:1],
                    in1=qm_col[:, r:r + 1], op0=ADD, op1=MULT,
                )
                nc.vector.tensor_scalar_add(out=bias, in0=bias, scalar1=-NEG_SOFT)

                st["Ub"], st["uwq"], st["bias"] = Ub, uwq, bias
                rt_raw = rowp.tile([128, NCHUNK], BF, tag="rt_raw")
                rt = rowp.tile([128, NCHUNK], BF, tag="rt")
                st["rt_raw"], st["rt"] = rt_raw, rt
                st["Hb"] = [None] * NPAIR
                st["ps_s"] = [None] * NPAIR
                st["eT"] = [None] * NPAIR
                return st

            # ---- pair-granular chunk stages (pair p covers chunks 2p, 2p+1) --

            def headA(st, p):
                g = st["g"]
                gv = g.rearrange("p (c gg) -> p c gg", gg=DG)
                Hb = hbp.tile([128, 2 * D2], BF, tag="hb")
                st["Hb"][p] = Hb
                nc.scalar.copy(
                    out=Hb.rearrange("q (k d) -> q k d", d=D2),
                    in_=gv[:, 2 * p:2 * p + 2, 0:D2],
                )

            def headB(st, p):
                Hb = st["Hb"][p]
                # trc cols: [0:128]=T1(2p), [128:256]=T1(2p+1),
                #           [256:384]=T2(2p), [384:512]=T2(2p+1)
                trc = ptrp.tile([KH, 512], BF, tag="tr")
                nc.tensor.transpose(trc[:, 0:128], Hb[:, 0:KH], identb)
                nc.tensor.transpose(trc[:, 128:256], Hb[:, D2:D2 + KH], identb)
                nc.tensor.transpose(trc[:, 256:384], Hb[:, KH:D2], identb)
                nc.tensor.transpose(trc[:, 384:512], Hb[:, D2 + KH:2 * D2], identb)
                ht = chp.tile([KH, 512], BF, tag="ht")
                nc.vector.tensor_copy(out=ht, in_=trc)
                ps_s = ps_sp.tile([J, 256], FP, tag="s")
                st["ps_s"][p] = ps_s
                nc.tensor.matmul(
                    ps_s, st["uwq"][:, 0:J], ht[:, 0:256], start=True, stop=False
                )
                nc.tensor.matmul(
                    ps_s, st["uwq"][:, J:2 * J], ht[:, 256:512],
                    start=False, stop=True,
                )

            def soft(st, p):
                eT = chp.tile([J, 256], BF, tag="eT")
                st["eT"][p] = eT
                nc.scalar.activation(
                    out=eT, in_=st["ps_s"][p], func=EXP,
                    bias=st["bias"][:, 0:1], scale=1.0,
                )

            def tailA(st, p):
                eT = st["eT"][p]
                psU = ps_up.tile([128, 2 * (D2 + 1)], FP, tag="u")
                st["psU"] = psU
                nc.tensor.matmul(
                    psU[:, 0:D2 + 1], eT[:, 0:128], st["Ub"],
                    start=True, stop=True,
                )
                nc.tensor.matmul(
                    psU[:, D2 + 1:2 * (D2 + 1)], eT[:, 128:256], st["Ub"],
                    start=True, stop=True,
                )
                eP = ps_ep.tile([128, 128], BF, tag="e")
                st["eP"] = eP
                nc.tensor.transpose(eP[:, 0:J], eT[:, 0:128], identb[0:J, 0:J])
                nc.tensor.transpose(eP[:, J:2 * J], eT[:, 128:256], identb[0:J, 0:J])

            def tailB(st, p):
                g = st["g"]
                gv = g.rearrange("p (c gg) -> p c gg", gg=DG)
                psU = st["psU"]
                psUv = psU.rearrange("q (k u) -> q k u", u=D2 + 1)
                rp = chp.tile([128, 2], FP, tag="rp")
                nc.vector.reciprocal(
                    out=rp.rearrange("q (k o) -> q k o", o=1),
                    in_=psUv[:, :, D2:D2 + 1],
                )
                # U_ = (e@U) * 1/denom on DVE (per-chunk scale varies within
                # the pair, so ACT's per-partition scale operand can't do it)
                nc.vector.tensor_tensor(
                    out=gv[:, 2 * p:2 * p + 2, D2:2 * D2],
                    in0=psUv[:, :, 0:D2],
                    in1=rp.rearrange("q (k o) -> q k o", o=1).broadcast_to(
                        [128, 2, D2]
                    ),
                    op=MULT,
                )
                ePv = st["eP"].rearrange("q (k j) -> q k j", j=J)
                nc.vector.reduce_max(
                    st["rt_raw"][:, 2 * p:2 * p + 2], ePv, axis=AXX
                )
                nc.vector.tensor_tensor(
                    out=gv[:, 2 * p:2 * p + 2, 2 * D2:3 * D2],
                    in0=gv[:, 2 * p:2 * p + 2, 0:D2],
                    in1=gv[:, 2 * p:2 * p + 2, D2:2 * D2], op=MULT,
                )

            def rowend(st):
                r = st["r"]
                # rt = rt_raw * cm (strided per-row view of cmT), one op
                rt = st["rt"]
                nc.vector.tensor_tensor(
                    out=rt.rearrange("q (c o) -> q c o", o=1),
                    in0=st["rt_raw"].rearrange("q (c o) -> q c o", o=1),
                    in1=cmT.rearrange("q (c rr) -> q c rr", rr=B)[:, :, r:r + 1],
                    op=MULT,
                )
                ps_h = rowps.tile([KH, 2], FP, tag="row")
                for p in range(NPAIR):
                    for k in range(2):
                        c = 2 * p + k
                        nc.tensor.matmul(
                            ps_h[:, 0:1], st["Hb"][p][:, k * D2:k * D2 + KH],
                            rt[:, c:c + 1],
                            start=(c == 0), stop=(c == NCHUNK - 1),
                        )
                for p in range(NPAIR):
                    for k in range(2):
                        c = 2 * p + k
                        nc.tensor.matmul(
                            ps_h[:, 1:2], st["Hb"][p][:, k * D2 + KH:(k + 1) * D2],
                            rt[:, c:c + 1],
                            start=(c == 0), stop=(c == NCHUNK - 1),
                        )
                rtp = rowp.tile([128, 1], FP, tag="rtp")
                nc.vector.reduce_sum(rtp, rt, axis=AXX)
                hbc = rowp.tile([KH, 2], BF, tag="hbc")
                nc.vector.tensor_copy(out=hbc, in_=ps_h)
                rtpT = rowps.tile([1, 128], FP, tag="row")
                nc.tensor.transpose(rtpT, rtp, identf)
                rtr = rowp.tile([1, 128], FP, tag="rtr")
                nc.vector.tensor_copy(out=rtr, in_=rtpT)
                rs = rowp.tile([1, 1], FP, tag="rs")
                nc.vector.reduce_sum(rs, rtr, axis=AXX)
                nc.vector.tensor_scalar_add(out=rs, in0=rs, scalar1=1e-13)
                nc.vector.reciprocal(out=rs, in_=rs)
                hbrow = rowps.tile([1, D2], BF, tag="row")
                nc.tensor.transpose(
                    hbrow[0:1, 0:KH], hbc[:, 0:1], identb[0:KH, 0:KH]
                )
                nc.tensor.transpose(
                    hbrow[0:1, KH:D2], hbc[:, 1:2], identb[0:KH, 0:KH]
                )
                hbar_sb = rowp.tile([1, D2], BF, tag="hbar_sb")
                nc.vector.tensor_scalar_mul(
                    out=hbar_sb, in0=hbrow, scalar1=rs[:, 0:1]
                )
                psb = rowps.tile([128, D2], FP, tag="row")
                nc.tensor.matmul(psb, ones_row, hbar_sb, start=True, stop=True)
                hb_sb = rowp.tile([128, D2], FP, tag="hb_sb")
                nc.vector.tensor_copy(out=hb_sb, in_=psb)
                st["hb_sb"] = hb_sb

            def rowfin(st, q):
                # H*H_ for chunk quad q (chunks 4q..4q+3) on GpSimd
                g = st["g"]
                gv = g.rearrange("p (c gg) -> p c gg", gg=DG)
                nc.gpsimd.tensor_mul(
                    gv[:, 4 * q:4 * q + 4, 3 * D2:4 * D2],
                    gv[:, 4 * q:4 * q + 4, 0:D2],
                    st["hb_sb"][:, None, :].broadcast_to([128, 4, D2]),
                )

            def head(st, p):
                headA(st, p)
                headB(st, p)

            # ---- cross-row pipelined schedule (pair-granular) ----
            grows = [None] * B
            for r in range(min(3, B)):
                grows[r] = load_grow(r)
            states = [None] * B
            states[0] = row_setup(0)
            states[0]["g"] = grows[0]
            head(states[0], 0)
            head(states[0], 1)
            soft(states[0], 0)
            for r in range(B):
                st = states[r]
                prev = states[r - 1] if r > 0 else None
                for p in range(NPAIR):
                    if p + 1 < NPAIR:
                        soft(st, p + 1)
                    tailA(st, p)
                    tailB(st, p)
                    if p + 2 < NPAIR:
                        head(st, p + 2)
                    if prev is not None and p % 2 == 0:
                        rowfin(prev, p // 2)
                    if p == 1 and r + 1 < B:
                        states[r + 1] = row_setup(r + 1)
                        states[r + 1]["g"] = grows[r + 1]
                    if p == NPAIR - 1:
                        if prev is not None:
                            store_grow(r - 1, prev["g"])
                        if r + 2 < B and grows[r + 2] is None:
                            grows[r + 2] = load_grow(r + 2)
                if r + 1 < B:
                    nxt = states[r + 1]
                    head(nxt, 0)
                rowend(st)
                if r + 1 < B:
                    nxt = states[r + 1]
                    head(nxt, 1)
                    soft(nxt, 0)
            for q in range(2):
                rowfin(states[B - 1], q)
            store_grow(B - 1, states[B - 1]["g"])

    if split_waits:
        _split_overwide_waits(nc)
    return nc


_NC_CACHE = None


def _get_nc():
    global _NC_CACHE
    if _NC_CACHE is None:
        _NC_CACHE = build_program()
    return _NC_CACHE


def run_sharded(inputs, trace=False):
    from concourse.bass_utils import run_bass_kernel_spmd

    H = np.ascontiguousarray(np.asarray(inputs["H"], dtype=np.float32))
    U = np.ascontiguousarray(np.asarray(inputs["U"], dtype=np.float32))
    cm = np.ascontiguousarray(np.asarray(inputs["c_mask"], dtype=np.float32))
    qm = np.ascontiguousarray(np.asarray(inputs["q_mask"], dtype=np.float32))
    w = np.ascontiguousarray(np.asarray(inputs["w"], dtype=np.float32))
    b = np.asarray(inputs["b"], dtype=np.float32).reshape(1, 1)

    nc = _get_nc()
    in_maps = []
    for c in range(N_CORES):
        s = slice(c * B, (c + 1) * B)
        in_maps.append(
            {"H": H[s], "U": U[s], "c_mask": cm[s], "q_mask": qm[s], "w": w, "b": b}
        )
    res = run_bass_kernel_spmd(
        nc, in_maps, core_ids=list(range(N_CORES)), trace=trace
    )
    G = np.concatenate([res.results[c]["G"] for c in range(N_CORES)], axis=0)
    return G, res


def kernel(H, U, c_mask, q_mask, w, b):
    G, _ = run_sharded(
        {"H": H, "U": U, "c_mask": c_mask, "q_mask": q_mask, "w": w, "b": b}
    )
    return G


# revision 13
# speedup vs baseline: 1.6482x; 1.3898x over previous
"""BiAttentionLayer Trainium2 kernel (Bass/Tile), data-parallel over batch N.

Full inputs:  H [64,1024,200], U [64,64,200], c_mask [64,1024],
              q_mask [64,64], w [600], b []
Full output:  G [64,1024,800] = concat([H, U_, H*U_, H*H_], -1)

Sharding: batch rows 8 per core across 8 NeuronCores; masks/w/b replicated.

Math (matches the reference to bf16 rounding; gate is rel_err < 2e-2):
  S = (H@w_h)[:,:,None] + (U@w_u)[:,None,:] + (H*w_hu)@U^T + b
  masked_softmax(v,m) == exp(v*m)*m / sum_j(...); the C2Q normalization is
  invariant to any per-t factor, so with NEG=100:
    e[t,j] = exp((S[t,j]+100)*qm[j] - 100)  (masked lanes underflow to ~0)
    U_ = (e @ U) / sum_j e,   rt = max_j(e)*cm,  a = rt/sum_t rt, H_ = a@H.

S is computed TRANSPOSED: S'[j,t] = sum_d uwq[d,j] * H^T[d,t] with
  uwq[d,j] = (w_hu[d]*U[j,d] + w_h[d]) * qm[j]          (folds S1, S3, mask)
  bias[j]  = (S2[j] + b + 100) * qm[j] - 100            (ACT exp bias column)
so e'[j,t] = exp(S' + bias) is ONE activation op and e' is directly the lhsT
of the U_ matmul; a small PE transpose recovers [t,j] just for the row max.
All PE operands are bf16; PSUM accumulation and H/U_/G stay fp32.

Perf notes driving this shape (HW-measured): every DVE/ACT instruction costs
~350-600ns of overhead regardless of width, and the PE runs at the 1.2GHz
throttled clock, so the kernel minimizes INSTRUCTION COUNT above all:
 - chunks are processed in QUADS (512 t-rows per instruction where possible:
   one Hb cast, one ht drain, one exp, one reduce_max, one H*U_ per quad)
 - all per-row setup (U^T, uwq weights, S2/bias columns, masks) is batched
   into a handful of whole-problem instructions at kernel start
 - the contraction splits d as 0:128 / 72:200 so every transpose and hbar
   matmul is a full 128-col weight load (FWL-eligible); the overlapping
   d-range 72:128 of block 2 is zeroed in the uwq weights via a mask column
 - DMA: one 819KB H load per row (ACT HWDGE ring) and two 1.6MB half-row G
   stores (SP ring) so loads overlap stores; 4 row buffers make the
   write-after-read wait on a reused buffer ~0.
"""

import os
import sys

for _p in ("/opt/trn_rl_repo", "/root/.axon_site/_ro/trn_rl_repo"):
    if os.path.isdir(_p) and _p not in sys.path:
        sys.path.insert(0, _p)

import numpy as np

import concourse.bass as bass
import concourse.tile as tile
from concourse import mybir
from concourse.masks import make_identity

N_CORES = 8
N_FULL = 64
B = N_CORES and N_FULL // N_CORES   # 8 batch rows per core
T = 1024
J = 64
D2 = 200
DG = 4 * D2                    # 800
NCHUNK = T // 128              # 8
NEG_SOFT = 100.0               # exp(x - 100): masked lanes underflow to ~0
KO = 72                        # block-2 d-offset: block1 = d 0:128, block2 = d 72:200

FP = mybir.dt.float32
BF = mybir.dt.bfloat16
MULT = mybir.AluOpType.mult
ADD = mybir.AluOpType.add
AXX = mybir.AxisListType.X
EXP = mybir.ActivationFunctionType.Exp
COPYF = mybir.ActivationFunctionType.Copy


def _split_overwide_waits(nc, max_waits=1):
    """This walrus build only encodes one semaphore wait per instruction;
    hoist extra waits onto no-ops just before the offending instruction."""
    for bb in nc.m.functions[0].blocks:
        i = 0
        while i < len(bb.instructions):
            ins = bb.instructions[i]
            si = getattr(ins, "sync_info", None)
            if si is not None and si.on_wait is not None and len(si.on_wait) > max_waits:
                waits = list(si.on_wait)
                si.on_wait = waits[-max_waits:]
                rest = waits[:-max_waits]
                k = 0
                while rest:
                    chunk, rest = rest[:max_waits], rest[max_waits:]
                    nop = mybir.InstNoOp(
                        name=f"{ins.name}-wsplit{k}",
                        engine=ins.engine,
                        bass_nofuse=True,
                        sync_info=mybir.SyncInfo(on_wait=chunk, on_update=[]),
                    )
                    bb.instructions.insert(i, nop)
                    i += 1
                    k += 1
            i += 1


def build_program(split_waits=True):
    nc = bass.Bass()

    H_d = nc.dram_tensor("H", [B, T, D2], FP, kind="ExternalInput")
    U_d = nc.dram_tensor("U", [B, J, D2], FP, kind="ExternalInput")
    cm_d = nc.dram_tensor("c_mask", [B, T], FP, kind="ExternalInput")
    qm_d = nc.dram_tensor("q_mask", [B, J], FP, kind="ExternalInput")
    w_d = nc.dram_tensor("w", [3 * D2], FP, kind="ExternalInput")
    b_d = nc.dram_tensor("b", [1, 1], FP, kind="ExternalInput")
    G_d = nc.dram_tensor("G", [B, T, DG], FP, kind="ExternalOutput")

    with tile.TileContext(nc) as tc:
        with (
            tc.tile_pool(name="const", bufs=1) as constp,
            tc.tile_pool(name="row", bufs=2) as rowp,
            tc.tile_pool(name="grow", bufs=4) as growp,
            tc.tile_pool(name="hb", bufs=4) as hbp,
            tc.tile_pool(name="chunk", bufs=3) as chp,
            tc.tile_pool(name="ps_tr", bufs=2, space="PSUM") as ptrp,
            tc.tile_pool(name="ps_s", bufs=2, space="PSUM") as ps_sp,
            tc.tile_pool(name="ps_u", bufs=2, space="PSUM") as ps_up,
            tc.tile_pool(name="ps_e", bufs=1, space="PSUM") as ps_ep,
            tc.tile_pool(name="ps_row", bufs=1, space="PSUM") as rowps,
        ):
            # ================= constants & whole-problem setup =================
            identf = constp.tile([128, 128], FP)
            make_identity(nc, identf)
            identb = constp.tile([128, 128], BF)
            nc.vector.tensor_copy(out=identb, in_=identf)
            ones_row = constp.tile([1, 128], BF)
            nc.vector.memset(ones_row, 1.0)
            # zmask zeroes the duplicated d-range 72:128 in block-2 weights
            zmask = constp.tile([128, 1], FP)
            nc.vector.memset(zmask, 1.0)
            nc.vector.memset(zmask[0:128 - KO, 0:1], 0.0)

            b64 = constp.tile([J, 1], FP)
            nc.gpsimd.dma_start(out=b64, in_=b_d[:, :].partition_broadcast(J))
            b100 = constp.tile([J, 1], FP)
            nc.vector.tensor_scalar_add(out=b100, in0=b64, scalar1=NEG_SOFT)

            # w columns on the overlapped split: block1 = d 0:128, block2 = d 72:200
            wh1 = constp.tile([128, 1], FP)
            wh2 = constp.tile([128, 1], FP)
            whu1 = constp.tile([128, 1], FP)
            whu2 = constp.tile([128, 1], FP)
            wu1b = constp.tile([128, 1], BF)
            wu2b = constp.tile([128, 1], BF)
            for sb, lo in ((wh1, 0), (wh2, KO), (whu1, 2 * D2), (whu2, 2 * D2 + KO),
                           (wu1b, D2), (wu2b, D2 + KO)):
                nc.gpsimd.dma_start(out=sb, in_=w_d[lo:lo + 128].unsqueeze(1))
            # zero the duplicated d-range in the block-2 wu column (S2 matmul)
            nc.vector.tensor_scalar_mul(out=wu2b, in0=wu2b, scalar1=zmask[:, 0:1])

            qm_b = constp.tile([128, B * J], BF)     # [p, r*64+j] = qm[r, j]
            nc.gpsimd.dma_start(
                out=qm_b, in_=qm_d.rearrange("r j -> (r j)").partition_broadcast(128)
            )
            qm_col = constp.tile([J, B], FP)         # [j, r]
            nc.gpsimd.dma_start(out=qm_col, in_=qm_d.rearrange("r j -> j r"))
            cm8 = constp.tile([B, T], FP)            # raw [r, t]
            nc.sync.dma_start(out=cm8, in_=cm_d[:, :])
            U_all = constp.tile([J, B * D2], FP)     # [j, r*200+d]
            nc.sync.dma_start(
                out=U_all.rearrange("j (r d) -> j r d", d=D2),
                in_=U_d.rearrange("r j d -> j r d"),
            )

            # cmT[p, c*8+r] = c_mask[r, c*128+p]  via 8 small PE transposes
            cmT = constp.tile([128, NCHUNK * B], BF)
            cmps = rowps.tile([128, NCHUNK * B], FP, tag="row")
            for c in range(NCHUNK):
                nc.tensor.transpose(
                    cmps[:, c * B:(c + 1) * B],
                    cm8[:, c * 128:(c + 1) * 128], identf[0:B, 0:B]
                )
            nc.vector.tensor_copy(out=cmT, in_=cmps)

            # Ub_all: bf16 copy of U with a ones column per row (denominator)
            UB1 = D2 + 1
            Ub_all = constp.tile([J, B * UB1], BF)   # [j, r*201 + d], col 200 = 1
            Ub_v = Ub_all.rearrange("j (r u) -> j r u", u=UB1)
            nc.scalar.copy(
                out=Ub_v[:, :, 0:D2],
                in_=U_all.rearrange("j (r d) -> j r d", d=D2),
            )
            nc.vector.memset(Ub_v[:, :, D2:UB1], 1.0)

            # UT_all[d, r*128 + (blk*64 + j)] = U[r, j, dblk]  (16 transposes)
            UT_all = constp.tile([128, B * 128], BF)
            for half in range(2):
                utps = rowps.tile([128, 4 * 128], BF, tag="row")
                for i in range(4):
                    r = half * 4 + i
                    nc.tensor.transpose(
                        utps[:, i * 128:i * 128 + J],
                        Ub_all[:, r * UB1:r * UB1 + 128], identb[0:J, 0:J]
                    )
                    nc.tensor.transpose(
                        utps[:, i * 128 + J:(i + 1) * 128],
                        Ub_all[:, r * UB1 + KO:r * UB1 + D2], identb[0:J, 0:J]
                    )
                nc.vector.tensor_copy(
                    out=UT_all[:, half * 512:(half + 1) * 512], in_=utps
                )

            # uwq_all[d, r*128 + blk*64 + j] = (whu[d]*U^T + wh[d]) * qm[j]
            # (block 2 additionally zeroed on the duplicated d-range via zmask)
            uwq_all = constp.tile([128, B * 128], BF)
            uw3 = uwq_all.rearrange("d (r x) -> d r x", x=128)
            ut3 = UT_all.rearrange("d (r x) -> d r x", x=128)
            nc.vector.tensor_scalar(
                out=uw3[:, :, 0:J], in0=ut3[:, :, 0:J],
                scalar1=whu1[:, 0:1], scalar2=wh1[:, 0:1], op0=MULT, op1=ADD,
            )
            nc.vector.tensor_scalar(
                out=uw3[:, :, J:2 * J], in0=ut3[:, :, J:2 * J],
                scalar1=whu2[:, 0:1], scalar2=wh2[:, 0:1], op0=MULT, op1=ADD,
            )
            qm_bv = qm_b.rearrange("d (r j) -> d r j", j=J)
            nc.vector.tensor_tensor(
                out=uw3[:, :, 0:J], in0=uw3[:, :, 0:J],
                in1=qm_bv, op=MULT,
            )
            nc.vector.scalar_tensor_tensor(
                out=uw3[:, :, J:2 * J], in0=uw3[:, :, J:2 * J],
                scalar=zmask[:, 0:1], in1=qm_bv, op0=MULT, op1=MULT,
            )

            # S2_all[j, r] = U[r] @ w_u, then bias_all = (S2+b+100)*qm - 100
            S2ps = rowps.tile([J, B], FP, tag="row")
            for r in range(B):
                nc.tensor.matmul(
                    S2ps[:, r:r + 1], UT_all[:, r * 128:r * 128 + J], wu1b,
                    start=True, stop=False,
                )
                nc.tensor.matmul(
                    S2ps[:, r:r + 1], UT_all[:, r * 128 + J:(r + 1) * 128], wu2b,
                    start=False, stop=True,
                )
            bias_all = constp.tile([J, B], FP)
            nc.vector.scalar_tensor_tensor(
                out=bias_all, in0=S2ps, scalar=b100[:, 0:1],
                in1=qm_col, op0=ADD, op1=MULT,
            )
            nc.vector.tensor_scalar_add(
                out=bias_all, in0=bias_all, scalar1=-NEG_SOFT
            )

            # ================= per-row / per-quad stages =================

            def load_grow(r):
                # H loads ride the ACT HWDGE ring so they overlap stores
                g = growp.tile([128, NCHUNK * DG], FP, tag="g")
                gv = g.rearrange("p (c gg) -> p c gg", gg=DG)
                nc.scalar.dma_start(
                    out=gv[:, :, 0:D2],
                    in_=H_d[r].rearrange("(c p) d -> p c d", p=128),
                )
                return g

            def store_half(r, g, q):
                gd = G_d[r].rearrange("(c p) gg -> p c gg", p=128)
                gs = g.rearrange("p (c gg) -> p c gg", gg=DG)
                nc.sync.dma_start(
                    out=gd[:, 4 * q:4 * q + 4, :], in_=gs[:, 4 * q:4 * q + 4, :]
                )

            def headA(st, q):
                g = st["g"]
                gv = g.rearrange("p (c gg) -> p c gg", gg=DG)
                Hb = hbp.tile([128, 4 * D2], BF, tag="hb")
                st["Hb"][q] = Hb
                nc.scalar.copy(
                    out=Hb.rearrange("p (k d) -> p k d", d=D2),
                    in_=gv[:, 4 * q:4 * q + 4, 0:D2],
                )

            def headB(st, q):
                r = st["r"]
                Hb = st["Hb"][q]
                # trc cols: [k*128 : k*128+128] = block1 of chunk k (d 0:128),
                #           [512 + k*128 : ...] = block2 (d 72:200)
                trc = ptrp.tile([128, 1024], BF, tag="tr")
                for k in range(4):
                    nc.tensor.transpose(
                        trc[:, k * 128:(k + 1) * 128],
                        Hb[:, k * D2:k * D2 + 128], identb,
                    )
                for k in range(4):
                    nc.tensor.transpose(
                        trc[:, 512 + k * 128:512 + (k + 1) * 128],
                        Hb[:, k * D2 + KO:(k + 1) * D2], identb,
                    )
                ht = chp.tile([128, 1024], BF, tag="ht")
                nc.vector.tensor_copy(out=ht, in_=trc)
                ps_s = ps_sp.tile([J, 512], FP, tag="s")
                st["ps_s"][q] = ps_s
                nc.tensor.matmul(
                    ps_s, uwq_all[:, r * 128:r * 128 + J], ht[:, 0:512],
                    start=True, stop=False,
                )
                nc.tensor.matmul(
                    ps_s, uwq_all[:, r * 128 + J:(r + 1) * 128], ht[:, 512:1024],
                    start=False, stop=True,
                )

            def soft(st, q):
                r = st["r"]
                eT = chp.tile([J, 512], BF, tag="eT")
                st["eT"][q] = eT
                nc.scalar.activation(
                    out=eT, in_=st["ps_s"][q], func=EXP,
                    bias=bias_all[:, r:r + 1], scale=1.0,
                )

            def tail(st, q):
                r = st["r"]
                g = st["g"]
                gv = g.rearrange("p (c gg) -> p c gg", gg=DG)
                eT = st["eT"][q]
                Ub_r = Ub_all[:, r * UB1:(r + 1) * UB1]
                eP = ps_ep.tile([128, 4 * J], BF, tag="e")
                psUs = []
                for pair in range(2):
                    psU = ps_up.tile([128, 2 * UB1], FP, tag="u")
                    psUs.append(psU)
                    for kk in range(2):
                        k = 2 * pair + kk
                        nc.tensor.matmul(
                            psU[:, kk * UB1:(kk + 1) * UB1],
                            eT[:, k * 128:(k + 1) * 128], Ub_r,
                            start=True, stop=True,
                        )
                for k in range(4):
                    nc.tensor.transpose(
                        eP[:, k * J:(k + 1) * J],
                        eT[:, k * 128:(k + 1) * 128], identb[0:J, 0:J],
                    )
                for pair in range(2):
                    psU = psUs[pair]
                    psUv = psU.rearrange("p (kk u) -> p kk u", u=UB1)
                    rp = chp.tile([128, 2], FP, tag=f"rp{pair}")
                    nc.vector.reciprocal(
                        out=rp.rearrange("p (kk o) -> p kk o", o=1),
                        in_=psUv[:, :, D2:UB1],
                    )
                    c2 = 4 * q + 2 * pair
                    nc.vector.tensor_tensor(
                        out=gv[:, c2:c2 + 2, D2:2 * D2],
                        in0=psUv[:, :, 0:D2],
                        in1=rp.rearrange("p (kk o) -> p kk o", o=1).broadcast_to(
                            [128, 2, D2]),
                        op=MULT,
                    )
                nc.vector.reduce_max(
                    st["rt_raw"][:, 4 * q:4 * q + 4],
                    eP.rearrange("p (k j) -> p k j", j=J), axis=AXX,
                )
                nc.vector.tensor_tensor(
                    out=gv[:, 4 * q:4 * q + 4, 2 * D2:3 * D2],
                    in0=gv[:, 4 * q:4 * q + 4, 0:D2],
                    in1=gv[:, 4 * q:4 * q + 4, D2:2 * D2], op=MULT,
                )

            def rowend(st):
                r = st["r"]
                rt = st["rt"]
                nc.vector.tensor_tensor(
                    out=rt.rearrange("p (c o) -> p c o", o=1),
                    in0=st["rt_raw"].rearrange("p (c o) -> p c o", o=1),
                    in1=cmT.rearrange("p (c rr) -> p c rr", rr=B)[:, :, r:r + 1],
                    op=MULT,
                )
                # hbar^T columns: col0 = d 0:128, col1 = d 72:200 (rows 56:128
                # hold d 128:200; rows 0:56 are computed but unused)
                ps_h = rowps.tile([128, 2], FP, tag="row")
                for blk in range(2):
                    off = 0 if blk == 0 else KO
                    for q in range(2):
                        for k in range(4):
                            c = 4 * q + k
                            nc.tensor.matmul(
                                ps_h[:, blk:blk + 1],
                                st["Hb"][q][:, k * D2 + off:k * D2 + off + 128],
                                rt[:, c:c + 1],
                                start=(c == 0), stop=(c == NCHUNK - 1),
                            )
                rtp = rowp.tile([128, 1], FP, tag="rtp")
                nc.vector.reduce_sum(rtp, rt, axis=AXX)
                hbc = rowp.tile([128, 2], BF, tag="hbc")
                nc.scalar.copy(out=hbc, in_=ps_h)
                rtpT = rowps.tile([1, 128], FP, tag="row")
                nc.tensor.transpose(rtpT, rtp, identf)
                rs = rowp.tile([1, 1], FP, tag="rs")
                nc.vector.reduce_sum(rs, rtpT, axis=AXX)
                nc.vector.reciprocal(out=rs, in_=rs)
                hbrow = rowps.tile([1, 256], BF, tag="row")
                nc.tensor.transpose(hbrow[0:1, 0:128], hbc[:, 0:1], identb)
                nc.tensor.transpose(hbrow[0:1, 128:256], hbc[:, 1:2], identb)
                hbar_sb = rowp.tile([1, D2], BF, tag="hbar_sb")
                nc.scalar.activation(
                    out=hbar_sb[0:1, 0:128], in_=hbrow[0:1, 0:128],
                    func=COPYF, scale=rs[:, 0:1],
                )
                nc.scalar.activation(
                    out=hbar_sb[0:1, 128:D2], in_=hbrow[0:1, 184:256],
                    func=COPYF, scale=rs[:, 0:1],
                )
                psb = rowps.tile([128, D2], FP, tag="row")
                nc.tensor.matmul(psb, ones_row, hbar_sb, start=True, stop=True)
                hb_sb = rowp.tile([128, D2], FP, tag="hb_sb")
                nc.scalar.copy(out=hb_sb, in_=psb)
                st["hb_sb"] = hb_sb

            def rowfin(st, q):
                g = st["g"]
                gv = g.rearrange("p (c gg) -> p c gg", gg=DG)
                nc.gpsimd.tensor_mul(
                    gv[:, 4 * q:4 * q + 4, 3 * D2:4 * D2],
                    gv[:, 4 * q:4 * q + 4, 0:D2],
                    st["hb_sb"][:, None, :].broadcast_to([128, 4, D2]),
                )

            def new_state(r, g):
                rt_raw = rowp.tile([128, NCHUNK], BF, tag="rt_raw")
                rt = rowp.tile([128, NCHUNK], BF, tag="rt")
                return {
                    "r": r, "g": g, "rt_raw": rt_raw, "rt": rt,
                    "Hb": [None, None], "ps_s": [None, None],
                    "eT": [None, None],
                }

            # ================= cross-row pipelined schedule =================
            grows = [None] * B
            for r in range(min(4, B)):
                grows[r] = load_grow(r)
            states = [None] * B
            states[0] = new_state(0, grows[0])
            headA(states[0], 0)
            headB(states[0], 0)
            headA(states[0], 1)
            headB(states[0], 1)
            soft(states[0], 0)
            for r in range(B):
                st = states[r]
                prev = states[r - 1] if r > 0 else None
                # ---- quad 0 ----
                soft(st, 1)
                tail(st, 0)
                if r + 1 < B:
                    if states[r + 1] is None:
                        states[r + 1] = new_state(r + 1, grows[r + 1])
                    headA(states[r + 1], 0)
                    headB(states[r + 1], 0)
                if prev is not None:
                    rowfin(prev, 0)
                    store_half(r - 1, prev["g"], 0)
                if r + 2 < B and grows[r + 2] is None:
                    grows[r + 2] = load_grow(r + 2)
                # ---- quad 1 ----
                tail(st, 1)
                if r + 1 < B:
                    headA(states[r + 1], 1)
                    headB(states[r + 1], 1)
                if prev is not None:
                    rowfin(prev, 1)
                    store_half(r - 1, prev["g"], 1)
                rowend(st)
                if r + 1 < B:
                    soft(states[r + 1], 0)
            last = states[B - 1]
            rowfin(last, 0)
            store_half(B - 1, last["g"], 0)
            rowfin(last, 1)
            store_half(B - 1, last["g"], 1)

    if split_waits:
        _split_overwide_waits(nc)
    return nc


_NC_CACHE = None


def _get_nc():
    global _NC_CACHE
    if _NC_CACHE is None:
        _NC_CACHE = build_program()
    return _NC_CACHE


def run_sharded(inputs, trace=False):
    from concourse.bass_utils import run_bass_kernel_spmd

    H = np.ascontiguousarray(np.asarray(inputs["H"], dtype=np.float32))
    U = np.ascontiguousarray(np.asarray(inputs["U"], dtype=np.float32))
    cm = np.ascontiguousarray(np.asarray(inputs["c_mask"], dtype=np.float32))
    qm = np.ascontiguousarray(np.asarray(inputs["q_mask"], dtype=np.float32))
    w = np.ascontiguousarray(np.asarray(inputs["w"], dtype=np.float32))
    b = np.asarray(inputs["b"], dtype=np.float32).reshape(1, 1)

    nc = _get_nc()
    in_maps = []
    for c in range(N_CORES):
        s = slice(c * B, (c + 1) * B)
        in_maps.append(
            {"H": H[s], "U": U[s], "c_mask": cm[s], "q_mask": qm[s], "w": w, "b": b}
        )
    res = run_bass_kernel_spmd(
        nc, in_maps, core_ids=list(range(N_CORES)), trace=trace
    )
    G = np.concatenate([res.results[c]["G"] for c in range(N_CORES)], axis=0)
    return G, res


def kernel(H, U, c_mask, q_mask, w, b):
    G, _ = run_sharded(
        {"H": H, "U": U, "c_mask": c_mask, "q_mask": q_mask, "w": w, "b": b}
    )
    return G


# revision 17
# speedup vs baseline: 1.7200x; 1.0436x over previous
"""BiAttentionLayer Trainium2 kernel (Bass/Tile), data-parallel over batch N.

Full inputs:  H [64,1024,200], U [64,64,200], c_mask [64,1024],
              q_mask [64,64], w [600], b []
Full output:  G [64,1024,800] = concat([H, U_, H*U_, H*H_], -1)

Sharding: batch rows 8 per core across 8 NeuronCores; masks/w/b replicated.

Math (matches the reference to bf16 rounding; gate is rel_err < 2e-2):
  S = (H@w_h)[:,:,None] + (U@w_u)[:,None,:] + (H*w_hu)@U^T + b
  masked_softmax(v,m) == exp(v*m)*m / sum_j(...); the C2Q normalization is
  invariant to any per-t factor, so with NEG=100:
    e[t,j] = exp((S[t,j]+100)*qm[j] - 100)  (masked lanes underflow to ~0)
    U_ = (e @ U) / sum_j e,   rt = max_j(e)*cm,  a = rt/sum_t rt, H_ = a@H.

S is computed TRANSPOSED: S'[j,t] = sum_d uwq[d,j] * H^T[d,t] with
  uwq[d,j] = (w_hu[d]*U[j,d] + w_h[d]) * qm[j]          (folds S1, S3, mask)
  bias[j]  = (S2[j] + b + 100) * qm[j] - 100            (ACT exp bias column)
so e'[j,t] = exp(S' + bias) is ONE activation op and e' is directly the lhsT
of the U_ matmul; a small PE transpose recovers [t,j] just for the row max.
All PE operands are bf16; PSUM accumulation and H/U_/G stay fp32.

Perf notes driving this shape (HW-measured): every DVE/ACT instruction costs
~350-600ns of overhead regardless of width, and the PE runs at the 1.2GHz
throttled clock, so the kernel minimizes INSTRUCTION COUNT above all:
 - chunks are processed in QUADS (512 t-rows per instruction where possible:
   one Hb cast, one ht drain, one exp, one reduce_max, one H*U_ per quad)
 - all per-row setup (U^T, uwq weights, S2/bias columns, masks) is batched
   into a handful of whole-problem instructions at kernel start
 - the contraction splits d as 0:128 / 72:200 so every transpose and hbar
   matmul is a full 128-col weight load (FWL-eligible); the overlapping
   d-range 72:128 of block 2 is zeroed in the uwq weights via a mask column
 - DMA: one 819KB H load per row (ACT HWDGE ring) and two 1.6MB half-row G
   stores (SP ring) so loads overlap stores; 4 row buffers make the
   write-after-read wait on a reused buffer ~0.
"""

import os
import sys

for _p in ("/opt/trn_rl_repo", "/root/.axon_site/_ro/trn_rl_repo"):
    if os.path.isdir(_p) and _p not in sys.path:
        sys.path.insert(0, _p)

import numpy as np

import concourse.bass as bass
import concourse.tile as tile
from concourse import mybir
from concourse.masks import make_identity

N_CORES = 8
N_FULL = 64
B = N_CORES and N_FULL // N_CORES   # 8 batch rows per core
T = 1024
J = 64
D2 = 200
DG = 4 * D2                    # 800
NCHUNK = T // 128              # 8
NEG_SOFT = 100.0               # exp(x - 100): masked lanes underflow to ~0
KO = 72                        # block-2 d-offset: block1 = d 0:128, block2 = d 72:200

FP = mybir.dt.float32
BF = mybir.dt.bfloat16
MULT = mybir.AluOpType.mult
ADD = mybir.AluOpType.add
AXX = mybir.AxisListType.X
EXP = mybir.ActivationFunctionType.Exp
COPYF = mybir.ActivationFunctionType.Copy


def _split_overwide_waits(nc, max_waits=1):
    """This walrus build only encodes one semaphore wait per instruction;
    hoist extra waits onto no-ops just before the offending instruction."""
    for bb in nc.m.functions[0].blocks:
        i = 0
        while i < len(bb.instructions):
            ins = bb.instructions[i]
            si = getattr(ins, "sync_info", None)
            if si is not None and si.on_wait is not None and len(si.on_wait) > max_waits:
                waits = list(si.on_wait)
                si.on_wait = waits[-max_waits:]
                rest = waits[:-max_waits]
                k = 0
                while rest:
                    chunk, rest = rest[:max_waits], rest[max_waits:]
                    nop = mybir.InstNoOp(
                        name=f"{ins.name}-wsplit{k}",
                        engine=ins.engine,
                        bass_nofuse=True,
                        sync_info=mybir.SyncInfo(on_wait=chunk, on_update=[]),
                    )
                    bb.instructions.insert(i, nop)
                    i += 1
                    k += 1
            i += 1


def build_program(split_waits=True):
    nc = bass.Bass()

    H_d = nc.dram_tensor("H", [B, T, D2], FP, kind="ExternalInput")
    U_d = nc.dram_tensor("U", [B, J, D2], FP, kind="ExternalInput")
    cm_d = nc.dram_tensor("c_mask", [B, T], FP, kind="ExternalInput")
    qm_d = nc.dram_tensor("q_mask", [B, J], FP, kind="ExternalInput")
    w_d = nc.dram_tensor("w", [3 * D2], FP, kind="ExternalInput")
    b_d = nc.dram_tensor("b", [1, 1], FP, kind="ExternalInput")
    G_d = nc.dram_tensor("G", [B, T, DG], FP, kind="ExternalOutput")

    with tile.TileContext(nc) as tc:
        with (
            tc.tile_pool(name="const", bufs=1) as constp,
            tc.tile_pool(name="row", bufs=2) as rowp,
            tc.tile_pool(name="grow", bufs=4) as growp,
            tc.tile_pool(name="hb", bufs=4) as hbp,
            tc.tile_pool(name="chunk", bufs=3) as chp,
            tc.tile_pool(name="ps_tr", bufs=2, space="PSUM") as ptrp,
            tc.tile_pool(name="ps_s", bufs=2, space="PSUM") as ps_sp,
            tc.tile_pool(name="ps_u", bufs=1, space="PSUM") as ps_up,
            tc.tile_pool(name="ps_e", bufs=1, space="PSUM") as ps_ep,
            tc.tile_pool(name="ps_row", bufs=1, space="PSUM") as rowps,
        ):
            # ================= constants & whole-problem setup =================
            identf = constp.tile([128, 128], FP)
            make_identity(nc, identf)
            identb = constp.tile([128, 128], BF)
            nc.vector.tensor_copy(out=identb, in_=identf)
            ones_row = constp.tile([1, 128], BF)
            nc.vector.memset(ones_row, 1.0)
            # zmask zeroes the duplicated d-range 72:128 in block-2 weights
            zmask = constp.tile([128, 1], FP)
            nc.vector.memset(zmask, 1.0)
            nc.vector.memset(zmask[0:128 - KO, 0:1], 0.0)

            b64 = constp.tile([J, 1], FP)
            nc.gpsimd.dma_start(out=b64, in_=b_d[:, :].partition_broadcast(J))
            b100 = constp.tile([J, 1], FP)
            nc.vector.tensor_scalar_add(out=b100, in0=b64, scalar1=NEG_SOFT)

            # w columns on the overlapped split: block1 = d 0:128, block2 = d 72:200
            wh1 = constp.tile([128, 1], FP)
            wh2 = constp.tile([128, 1], FP)
            whu1 = constp.tile([128, 1], FP)
            whu2 = constp.tile([128, 1], FP)
            wu1b = constp.tile([128, 1], BF)
            wu2b = constp.tile([128, 1], BF)
            for sb, lo in ((wh1, 0), (wh2, KO), (whu1, 2 * D2), (whu2, 2 * D2 + KO),
                           (wu1b, D2), (wu2b, D2 + KO)):
                nc.gpsimd.dma_start(out=sb, in_=w_d[lo:lo + 128].unsqueeze(1))
            # zero the duplicated d-range in the block-2 wu column (S2 matmul)
            nc.vector.tensor_scalar_mul(out=wu2b, in0=wu2b, scalar1=zmask[:, 0:1])

            qm_b = constp.tile([128, B * J], BF)     # [p, r*64+j] = qm[r, j]
            nc.gpsimd.dma_start(
                out=qm_b, in_=qm_d.rearrange("r j -> (r j)").partition_broadcast(128)
            )
            qm_col = constp.tile([J, B], FP)         # [j, r]
            nc.gpsimd.dma_start(out=qm_col, in_=qm_d.rearrange("r j -> j r"))
            cm8 = constp.tile([B, T], FP)            # raw [r, t]
            nc.sync.dma_start(out=cm8, in_=cm_d[:, :])
            U_all = constp.tile([J, B * D2], FP)     # [j, r*200+d]
            nc.sync.dma_start(
                out=U_all.rearrange("j (r d) -> j r d", d=D2),
                in_=U_d.rearrange("r j d -> j r d"),
            )

            # cmT[p, c*8+r] = c_mask[r, c*128+p]  via 8 small PE transposes
            cmT = constp.tile([128, NCHUNK * B], BF)
            cmps = rowps.tile([128, NCHUNK * B], FP, tag="row")
            for c in range(NCHUNK):
                nc.tensor.transpose(
                    cmps[:, c * B:(c + 1) * B],
                    cm8[:, c * 128:(c + 1) * 128], identf[0:B, 0:B]
                )
            nc.vector.tensor_copy(out=cmT, in_=cmps)

            # Ub_all: bf16 copy of U with a ones column per row (denominator)
            UB1 = D2 + 1
            Ub_all = constp.tile([J, B * UB1], BF)   # [j, r*201 + d], col 200 = 1
            Ub_v = Ub_all.rearrange("j (r u) -> j r u", u=UB1)
            nc.scalar.copy(
                out=Ub_v[:, :, 0:D2],
                in_=U_all.rearrange("j (r d) -> j r d", d=D2),
            )
            nc.vector.memset(Ub_v[:, :, D2:UB1], 1.0)

            # UT_all[d, r*128 + (blk*64 + j)] = U[r, j, dblk]  (16 transposes)
            UT_all = constp.tile([128, B * 128], BF)
            for half in range(2):
                utps = rowps.tile([128, 4 * 128], BF, tag="row")
                for i in range(4):
                    r = half * 4 + i
                    nc.tensor.transpose(
                        utps[:, i * 128:i * 128 + J],
                        Ub_all[:, r * UB1:r * UB1 + 128], identb[0:J, 0:J]
                    )
                    nc.tensor.transpose(
                        utps[:, i * 128 + J:(i + 1) * 128],
                        Ub_all[:, r * UB1 + KO:r * UB1 + D2], identb[0:J, 0:J]
                    )
                nc.vector.tensor_copy(
                    out=UT_all[:, half * 512:(half + 1) * 512], in_=utps
                )

            # uwq_all[d, r*128 + blk*64 + j] = (whu[d]*U^T + wh[d]) * qm[j]
            # (block 2 additionally zeroed on the duplicated d-range via zmask)
            uwq_all = constp.tile([128, B * 128], BF)
            uw3 = uwq_all.rearrange("d (r x) -> d r x", x=128)
            ut3 = UT_all.rearrange("d (r x) -> d r x", x=128)
            nc.vector.tensor_scalar(
                out=uw3[:, :, 0:J], in0=ut3[:, :, 0:J],
                scalar1=whu1[:, 0:1], scalar2=wh1[:, 0:1], op0=MULT, op1=ADD,
            )
            nc.vector.tensor_scalar(
                out=uw3[:, :, J:2 * J], in0=ut3[:, :, J:2 * J],
                scalar1=whu2[:, 0:1], scalar2=wh2[:, 0:1], op0=MULT, op1=ADD,
            )
            qm_bv = qm_b.rearrange("d (r j) -> d r j", j=J)
            nc.vector.tensor_tensor(
                out=uw3[:, :, 0:J], in0=uw3[:, :, 0:J],
                in1=qm_bv, op=MULT,
            )
            nc.vector.scalar_tensor_tensor(
                out=uw3[:, :, J:2 * J], in0=uw3[:, :, J:2 * J],
                scalar=zmask[:, 0:1], in1=qm_bv, op0=MULT, op1=MULT,
            )

            # S2_all[j, r] = U[r] @ w_u, then bias_all = (S2+b+100)*qm - 100
            S2ps = rowps.tile([J, B], FP, tag="row")
            for r in range(B):
                nc.tensor.matmul(
                    S2ps[:, r:r + 1], UT_all[:, r * 128:r * 128 + J], wu1b,
                    start=True, stop=False,
                )
                nc.tensor.matmul(
                    S2ps[:, r:r + 1], UT_all[:, r * 128 + J:(r + 1) * 128], wu2b,
                    start=False, stop=True,
                )
            bias_all = constp.tile([J, B], FP)
            nc.vector.scalar_tensor_tensor(
                out=bias_all, in0=S2ps, scalar=b100[:, 0:1],
                in1=qm_col, op0=ADD, op1=MULT,
            )
            nc.vector.tensor_scalar_add(
                out=bias_all, in0=bias_all, scalar1=-NEG_SOFT
            )

            # ================= per-row / per-quad stages =================

            def load_grow(r):
                # H loads ride the ACT HWDGE ring so they overlap stores
                g = growp.tile([128, NCHUNK * DG], FP, tag="g")
                gv = g.rearrange("p (c gg) -> p c gg", gg=DG)
                nc.scalar.dma_start(
                    out=gv[:, :, 0:D2],
                    in_=H_d[r].rearrange("(c p) d -> p c d", p=128),
                )
                return g

            def store_main(r, g, q):
                # H / U_ / H*U_ columns: ready right after tail(q), not gated
                # by the H_ reduction, so the store ring stays fed
                gd = G_d[r].rearrange("(c p) gg -> p c gg", p=128)
                gs = g.rearrange("p (c gg) -> p c gg", gg=DG)
                nc.sync.dma_start(
                    out=gd[:, 4 * q:4 * q + 4, 0:3 * D2],
                    in_=gs[:, 4 * q:4 * q + 4, 0:3 * D2],
                )

            def store_hh(r, g, q):
                gd = G_d[r].rearrange("(c p) gg -> p c gg", p=128)
                gs = g.rearrange("p (c gg) -> p c gg", gg=DG)
                nc.sync.dma_start(
                    out=gd[:, 4 * q:4 * q + 4, 3 * D2:DG],
                    in_=gs[:, 4 * q:4 * q + 4, 3 * D2:DG],
                )

            def headA(st, q):
                g = st["g"]
                gv = g.rearrange("p (c gg) -> p c gg", gg=DG)
                Hb = hbp.tile([128, 4 * D2], BF, tag="hb")
                st["Hb"][q] = Hb
                nc.scalar.copy(
                    out=Hb.rearrange("p (k d) -> p k d", d=D2),
                    in_=gv[:, 4 * q:4 * q + 4, 0:D2],
                )

            def headB(st, q):
                r = st["r"]
                Hb = st["Hb"][q]
                # trc cols: [k*128 : k*128+128] = block1 of chunk k (d 0:128),
                #           [512 + k*128 : ...] = block2 (d 72:200)
                trc = ptrp.tile([128, 1024], BF, tag="tr")
                for k in range(4):
                    nc.tensor.transpose(
                        trc[:, k * 128:(k + 1) * 128],
                        Hb[:, k * D2:k * D2 + 128], identb,
                    )
                for k in range(4):
                    nc.tensor.transpose(
                        trc[:, 512 + k * 128:512 + (k + 1) * 128],
                        Hb[:, k * D2 + KO:(k + 1) * D2], identb,
                    )
                ht = chp.tile([128, 1024], BF, tag="ht")
                nc.vector.tensor_copy(out=ht, in_=trc)
                ps_s = ps_sp.tile([J, 512], FP, tag="s")
                st["ps_s"][q] = ps_s
                nc.tensor.matmul(
                    ps_s, uwq_all[:, r * 128:r * 128 + J], ht[:, 0:512],
                    start=True, stop=False,
                )
                nc.tensor.matmul(
                    ps_s, uwq_all[:, r * 128 + J:(r + 1) * 128], ht[:, 512:1024],
                    start=False, stop=True,
                )

            def soft(st, q):
                r = st["r"]
                eT = chp.tile([J, 512], BF, tag="eT")
                st["eT"][q] = eT
                nc.scalar.activation(
                    out=eT, in_=st["ps_s"][q], func=EXP,
                    bias=bias_all[:, r:r + 1], scale=1.0,
                )

            def tail(st, q):
                r = st["r"]
                g = st["g"]
                gv = g.rearrange("p (c gg) -> p c gg", gg=DG)
                eT = st["eT"][q]
                Ub_r = Ub_all[:, r * UB1:(r + 1) * UB1]
                eP = ps_ep.tile([128, 4 * J], BF, tag="e")
                # one 2-bank psU tile, chunk regions at a uniform 256-f32
                # stride (201 used + pad) so none crosses a bank boundary and
                # reciprocal + U_ scale each run as ONE op per quad
                psU = ps_up.tile([128, 4 * 256], FP, tag="u")
                for k in range(4):
                    nc.tensor.matmul(
                        psU[:, k * 256:k * 256 + UB1],
                        eT[:, k * 128:(k + 1) * 128], Ub_r,
                        start=True, stop=True,
                    )
                for k in range(4):
                    nc.tensor.transpose(
                        eP[:, k * J:(k + 1) * J],
                        eT[:, k * 128:(k + 1) * 128], identb[0:J, 0:J],
                    )
                psUv = psU.rearrange("p (k u) -> p k u", u=256)
                rp = chp.tile([128, 4], FP, tag="rp")
                nc.vector.reciprocal(
                    out=rp.rearrange("p (k o) -> p k o", o=1),
                    in_=psUv[:, :, D2:UB1],
                )
                nc.vector.tensor_tensor(
                    out=gv[:, 4 * q:4 * q + 4, D2:2 * D2],
                    in0=psUv[:, :, 0:D2],
                    in1=rp.rearrange("p (k o) -> p k o", o=1).broadcast_to(
                        [128, 4, D2]),
                    op=MULT,
                )
                nc.vector.reduce_max(
                    st["rt_raw"][:, 4 * q:4 * q + 4],
                    eP.rearrange("p (k j) -> p k j", j=J), axis=AXX,
                )
                nc.vector.tensor_tensor(
                    out=gv[:, 4 * q:4 * q + 4, 2 * D2:3 * D2],
                    in0=gv[:, 4 * q:4 * q + 4, 0:D2],
                    in1=gv[:, 4 * q:4 * q + 4, D2:2 * D2], op=MULT,
                )

            def rowend(st):
                r = st["r"]
                rt = st["rt"]
                nc.vector.tensor_tensor(
                    out=rt.rearrange("p (c o) -> p c o", o=1),
                    in0=st["rt_raw"].rearrange("p (c o) -> p c o", o=1),
                    in1=cmT.rearrange("p (c rr) -> p c rr", rr=B)[:, :, r:r + 1],
                    op=MULT,
                )
                # hbar^T columns: col0 = d 0:128, col1 = d 72:200 (rows 56:128
                # hold d 128:200; rows 0:56 are computed but unused)
                ps_h = rowps.tile([128, 2], FP, tag="row")
                for blk in range(2):
                    off = 0 if blk == 0 else KO
                    for q in range(2):
                        for k in range(4):
                            c = 4 * q + k
                            nc.tensor.matmul(
                                ps_h[:, blk:blk + 1],
                                st["Hb"][q][:, k * D2 + off:k * D2 + off + 128],
                                rt[:, c:c + 1],
                                start=(c == 0), stop=(c == NCHUNK - 1),
                            )
                rtp = rowp.tile([128, 1], FP, tag="rtp")
                nc.vector.reduce_sum(rtp, rt, axis=AXX)
                hbc = rowp.tile([128, 2], BF, tag="hbc")
                nc.scalar.copy(out=hbc, in_=ps_h)
                rtpT = rowps.tile([1, 128], FP, tag="row")
                nc.tensor.transpose(rtpT, rtp, identf)
                rs = rowp.tile([1, 1], FP, tag="rs")
                nc.vector.reduce_sum(rs, rtpT, axis=AXX)
                nc.vector.reciprocal(out=rs, in_=rs)
                hbrow = rowps.tile([1, 256], BF, tag="row")
                nc.tensor.transpose(hbrow[0:1, 0:128], hbc[:, 0:1], identb)
                nc.tensor.transpose(hbrow[0:1, 128:256], hbc[:, 1:2], identb)
                hbar_sb = rowp.tile([1, D2], BF, tag="hbar_sb")
                nc.scalar.activation(
                    out=hbar_sb[0:1, 0:128], in_=hbrow[0:1, 0:128],
                    func=COPYF, scale=rs[:, 0:1],
                )
                nc.scalar.activation(
                    out=hbar_sb[0:1, 128:D2], in_=hbrow[0:1, 184:256],
                    func=COPYF, scale=rs[:, 0:1],
                )
                psb = rowps.tile([128, D2], FP, tag="row")
                nc.tensor.matmul(psb, ones_row, hbar_sb, start=True, stop=True)
                hb_sb = rowp.tile([128, D2], FP, tag="hb_sb")
                nc.scalar.copy(out=hb_sb, in_=psb)
                st["hb_sb"] = hb_sb

            def rowfin(st, q):
                g = st["g"]
                gv = g.rearrange("p (c gg) -> p c gg", gg=DG)
                nc.gpsimd.tensor_mul(
                    gv[:, 4 * q:4 * q + 4, 3 * D2:4 * D2],
                    gv[:, 4 * q:4 * q + 4, 0:D2],
                    st["hb_sb"][:, None, :].broadcast_to([128, 4, D2]),
                )

            def new_state(r, g):
                rt_raw = rowp.tile([128, NCHUNK], BF, tag="rt_raw")
                rt = rowp.tile([128, NCHUNK], BF, tag="rt")
                return {
                    "r": r, "g": g, "rt_raw": rt_raw, "rt": rt,
                    "Hb": [None, None], "ps_s": [None, None],
                    "eT": [None, None],
                }

            # ================= cross-row pipelined schedule =================
            grows = [None] * B
            for r in range(min(4, B)):
                grows[r] = load_grow(r)
            states = [None] * B
            states[0] = new_state(0, grows[0])
            headA(states[0], 0)
            headB(states[0], 0)
            headA(states[0], 1)
            headB(states[0], 1)
            soft(states[0], 0)
            for r in range(B):
                st = states[r]
                prev = states[r - 1] if r > 0 else None
                # ---- quad 0 ----
                soft(st, 1)
                tail(st, 0)
                if r + 1 < B:
                    if states[r + 1] is None:
                        states[r + 1] = new_state(r + 1, grows[r + 1])
                    headA(states[r + 1], 0)
                    headB(states[r + 1], 0)
                store_main(r, st["g"], 0)
                if prev is not None:
                    rowfin(prev, 0)
                if r + 2 < B and grows[r + 2] is None:
                    grows[r + 2] = load_grow(r + 2)
                # ---- quad 1 ----
                tail(st, 1)
                if r + 1 < B:
                    headA(states[r + 1], 1)
                    headB(states[r + 1], 1)
                store_main(r, st["g"], 1)
                if prev is not None:
                    rowfin(prev, 1)
                    store_hh(r - 1, prev["g"], 0)
                    store_hh(r - 1, prev["g"], 1)
                rowend(st)
                if r + 1 < B:
                    soft(states[r + 1], 0)
            # last row epilogue: H*H_ split across GpSimd and DVE so the
            # final stores start as early as possible
            last = states[B - 1]
            g = last["g"]
            gvl = g.rearrange("p (c gg) -> p c gg", gg=DG)
            hbb = last["hb_sb"][:, None, :]
            for p in range(4):
                eng = nc.gpsimd if p % 2 == 0 else nc.vector
                eng.tensor_mul(
                    gvl[:, 2 * p:2 * p + 2, 3 * D2:4 * D2],
                    gvl[:, 2 * p:2 * p + 2, 0:D2],
                    hbb.broadcast_to([128, 2, D2]),
                )
            store_hh(B - 1, g, 0)
            store_hh(B - 1, g, 1)

    if split_waits:
        _split_overwide_waits(nc)
    return nc


_NC_CACHE = None


def _get_nc():
    global _NC_CACHE
    if _NC_CACHE is None:
        _NC_CACHE = build_program()
    return _NC_CACHE


def run_sharded(inputs, trace=False):
    from concourse.bass_utils import run_bass_kernel_spmd

    H = np.ascontiguousarray(np.asarray(inputs["H"], dtype=np.float32))
    U = np.ascontiguousarray(np.asarray(inputs["U"], dtype=np.float32))
    cm = np.ascontiguousarray(np.asarray(inputs["c_mask"], dtype=np.float32))
    qm = np.ascontiguousarray(np.asarray(inputs["q_mask"], dtype=np.float32))
    w = np.ascontiguousarray(np.asarray(inputs["w"], dtype=np.float32))
    b = np.asarray(inputs["b"], dtype=np.float32).reshape(1, 1)

    nc = _get_nc()
    in_maps = []
    for c in range(N_CORES):
        s = slice(c * B, (c + 1) * B)
        in_maps.append(
            {"H": H[s], "U": U[s], "c_mask": cm[s], "q_mask": qm[s], "w": w, "b": b}
        )
    res = run_bass_kernel_spmd(
        nc, in_maps, core_ids=list(range(N_CORES)), trace=trace
    )
    G = np.concatenate([res.results[c]["G"] for c in range(N_CORES)], axis=0)
    return G, res


def kernel(H, U, c_mask, q_mask, w, b):
    G, _ = run_sharded(
        {"H": H, "U": U, "c_mask": c_mask, "q_mask": q_mask, "w": w, "b": b}
    )
    return G
